# revision 2
# baseline (speedup 1.0000x reference)
"""Trainium2 Bass kernel for greedy GRU decode (AnswerModule).

B=64, H=1024, V=50257 (padded 51200), T=20 steps, 8 NeuronCores.

Strategy (tensor-parallel over vocab):
 - W_out/b_out/word-emb-gather sharded over vocab (6400 rows/core).
 - Screen: bf16 matmul h @ W_out_shard.T (+bias via K=1 matmul row) -> fp32 psum.
 - top-8 via max8/max_index; top-4 rescored exactly in fp32 via indirect-DMA
   gather of [W|b] rows + tensor_tensor_reduce dots.
 - AllGather (val,idx) -> global argmax with lowest-index tie-break.
 - Embedding gather from a replicated table; GRU sharded over H (128 rows/core)
   in fp32; AllGather h chunks.
"""
import sys
import numpy as np

sys.path.insert(0, "/opt/trn_rl_repo")
sys.path.insert(0, "/root/.axon_site")

import ml_dtypes

B = 64
H = 1024
V = 50257
VPAD = 51200
VSH = VPAD // 8          # 6400
T = 20
NCORES = 8
NK = H // 128            # 8 contraction chunks
NV = VSH // 512          # 12.5 -> handle as 12 full + 1 half? use 400-col tiles instead
# use vtile size 512 with 12 full tiles + 1 tile of 256: 12*512+256 = 6400
VT_SIZES = [512] * 12 + [256]
KCAND = 4
BIG = float(1 << 24)
PAD_BIAS = -10000.0


def build(steps=T):
    import concourse.bass as bass
    import concourse.bacc as bacc
    import concourse.mybir as mybir
    from concourse import tile
    from concourse.tile_rust import add_dep_helper
    from concourse.masks import make_identity

    F32 = mybir.dt.float32
    BF16 = mybir.dt.bfloat16
    U32 = mybir.dt.uint32
    I32 = mybir.dt.int32
    AF = mybir.ActivationFunctionType
    ALU = mybir.AluOpType
    AX = mybir.AxisListType

    nc = bacc.Bacc("TRN2", target_bir_lowering=False, debug=False, num_devices=NCORES)

    # ---- external inputs (per-core shards prepared on host) ----
    wt_bf = nc.dram_tensor("wt_bf", [128, NK, VSH], BF16, kind="ExternalInput")
    bias_bf = nc.dram_tensor("bias_bf", [1, VSH], BF16, kind="ExternalInput")
    w_aug = nc.dram_tensor("w_aug", [VSH, 1032], F32, kind="ExternalInput")
    wemb = nc.dram_tensor("wemb", [V, 1024], F32, kind="ExternalInput")
    we_lhsT = nc.dram_tensor("we_lhsT", [128, 3, NK, 128], F32, kind="ExternalInput")
    whh_lhsT = nc.dram_tensor("whh_lhsT", [128, 3, NK, 128], F32, kind="ExternalInput")
    cT_in = nc.dram_tensor("cT_in", [128, 3, 64], F32, kind="ExternalInput")
    bhh_n_in = nc.dram_tensor("bhh_n_in", [128, 1], F32, kind="ExternalInput")
    h0_own_in = nc.dram_tensor("h0_own_in", [128, 64], F32, kind="ExternalInput")
    hT0_in = nc.dram_tensor("hT0_in", [128, NK, 64], F32, kind="ExternalInput")
    haug0_in = nc.dram_tensor("haug0_in", [64, 1032], F32, kind="ExternalInput")
    coff_in = nc.dram_tensor("coff_in", [64, 1], F32, kind="ExternalInput")

    out = nc.dram_tensor("out", [64, steps], I32, kind="ExternalOutput")

    # ---- collective DRAM buffers (double buffered) ----
    ag1_in = [nc.dram_tensor(f"ag1_in{i}", [64, 2], F32) for i in range(2)]
    ag1_out = [nc.dram_tensor(f"ag1_out{i}", [8, 64, 2], F32, addr_space="Shared") for i in range(2)]
    ag2_in = [nc.dram_tensor(f"ag2_in{i}", [128, 64], F32) for i in range(2)]
    ag2_out = [nc.dram_tensor(f"ag2_out{i}", [8, 128, 64], F32, addr_space="Shared") for i in range(2)]

    from contextlib import ExitStack
    ctx = ExitStack()
    with ctx:
        tc = ctx.enter_context(tile.TileContext(nc))

        # ---- sbuf tensors ----
        wt_sb = nc.alloc_sbuf_tensor("wt_sb", [128, NK, VSH], BF16)
        bias_sb = nc.alloc_sbuf_tensor("bias_sb", [1, VSH], BF16)
        ones_sb = nc.alloc_sbuf_tensor("ones_sb", [1, 64], BF16)
        we_sb = nc.alloc_sbuf_tensor("we_sb", [128, 3, NK, 128], F32)
        whh_sb = nc.alloc_sbuf_tensor("whh_sb", [128, 3, NK, 128], F32)
        cT_sb = nc.alloc_sbuf_tensor("cT_sb", [128, 3, 64], F32)
        bhhn_sb = nc.alloc_sbuf_tensor("bhhn_sb", [128, 1], F32)
        coff_sb = nc.alloc_sbuf_tensor("coff_sb", [64, 1], F32)
        ident64 = nc.alloc_sbuf_tensor("ident64", [64, 64], F32)
        ident128 = nc.alloc_sbuf_tensor("ident128", [128, 128], F32)

        hT = nc.alloc_sbuf_tensor("hT", [128, NK, 64], F32)
        hT_bf = nc.alloc_sbuf_tensor("hT_bf", [128, NK, 64], BF16)
        h_aug = nc.alloc_sbuf_tensor("h_aug", [64, 1032], F32)
        h_own = nc.alloc_sbuf_tensor("h_own", [128, 64], F32)
        hnew = nc.alloc_sbuf_tensor("hnew", [128, 64], F32)
        embT = nc.alloc_sbuf_tensor("embT", [128, NK, 64], F32)
        emb_sb = nc.alloc_sbuf_tensor("emb_sb", [64, 1024], F32)

        logits = nc.alloc_sbuf_tensor("logits", [64, VSH], F32)
        maxv = nc.alloc_sbuf_tensor("maxv", [64, 8], F32)
        maxi = nc.alloc_sbuf_tensor("maxi", [64, 8], U32)
        maxi_f = nc.alloc_sbuf_tensor("maxi_f", [64, KCAND], F32)
        g4 = nc.alloc_sbuf_tensor("g4", [64, KCAND, 1032], F32)
        resc = nc.alloc_sbuf_tensor("resc", [64, KCAND], F32)

        rmax = nc.alloc_sbuf_tensor("rmax", [64, 1], F32)
        rtmp = nc.alloc_sbuf_tensor("rtmp", [64, KCAND], F32)
        rmask = nc.alloc_sbuf_tensor("rmask", [64, KCAND], F32)
        lidx = nc.alloc_sbuf_tensor("lidx", [64, 1], F32)
        agin_sb = nc.alloc_sbuf_tensor("agin_sb", [64, 2], F32)
        gg = nc.alloc_sbuf_tensor("gg", [64, 8, 2], F32)
        gmax = nc.alloc_sbuf_tensor("gmax", [64, 1], F32)
        gmask = nc.alloc_sbuf_tensor("gmask", [64, 8], F32)
        gtmp = nc.alloc_sbuf_tensor("gtmp", [64, 8], F32)
        tokf = nc.alloc_sbuf_tensor("tokf", [64, 1], F32)
        toku = nc.alloc_sbuf_tensor("toku", [64, 1], U32)
        toks = nc.alloc_sbuf_tensor("toks", [64, steps], I32)

        r_sb = nc.alloc_sbuf_tensor("r_sb", [128, 64], F32)
        z_sb = nc.alloc_sbuf_tensor("z_sb", [128, 64], F32)
        n_sb = nc.alloc_sbuf_tensor("n_sb", [128, 64], F32)
        gt1 = nc.alloc_sbuf_tensor("gt1", [128, 64], F32)
        gt2 = nc.alloc_sbuf_tensor("gt2", [128, 64], F32)

        # ---- psum ----
        ps_scr = [ctx.enter_context(nc.psum_tensor(f"ps_scr{i}", [64, 512], F32)) for i in range(2)]
        ps_g = ctx.enter_context(nc.psum_tensor("ps_g", [128, 2, 64], F32))
        ps_ghn = ctx.enter_context(nc.psum_tensor("ps_ghn", [128, 64], F32))
        ps_gin = ctx.enter_context(nc.psum_tensor("ps_gin", [128, 64], F32))
        ps_e = ctx.enter_context(nc.psum_tensor("ps_e", [128, 512], F32))
        ps_h0 = ctx.enter_context(nc.psum_tensor("ps_h0", [64, 512], F32))
        ps_h1 = ctx.enter_context(nc.psum_tensor("ps_h1", [64, 512], F32))

        # ---- preamble: load everything ----
        nc.sync.dma_start(wt_sb[:], wt_bf[:])
        nc.sync.dma_start(bias_sb[:], bias_bf[:])
        nc.sync.dma_start(we_sb[:], we_lhsT[:])
        nc.sync.dma_start(whh_sb[:], whh_lhsT[:])
        nc.sync.dma_start(cT_sb[:], cT_in[:])
        nc.sync.dma_start(bhhn_sb[:], bhh_n_in[:])
        nc.sync.dma_start(coff_sb[:], coff_in[:])
        nc.sync.dma_start(h_own[:], h0_own_in[:])
        nc.sync.dma_start(hT[:], hT0_in[:])
        nc.sync.dma_start(h_aug[:], haug0_in[:])
        nc.vector.tensor_copy(hT_bf[:], hT[:])
        nc.vector.memset(ones_sb[:], 1.0)
        make_identity(nc, ident64[:])
        make_identity(nc, ident128[:])

        prev_gg_read = [None, None]   # for WAR dep two steps back (ag1)
        prev_hT_read = [None, None]   # (ag2)

        for t in range(steps):
            db = t % 2

            # ===== screen matmuls (bf16) + bias row =====
            voff = 0
            for vt, vsz in enumerate(VT_SIZES):
                ps = ps_scr[vt % 2]
                for k in range(NK):
                    nc.tensor.matmul(
                        ps[:, 0:vsz],
                        hT_bf[:, k, :],
                        wt_sb[:, k, voff:voff + vsz],
                        start=(k == 0), stop=False)
                nc.tensor.matmul(
                    ps[:, 0:vsz],
                    ones_sb[:],
                    bias_sb[:, voff:voff + vsz],
                    start=False, stop=True)
                nc.scalar.copy(logits[:, voff:voff + vsz], ps[:, 0:vsz])
                voff += vsz

            # ===== GRU h-side matmuls (only need hT) — emitted early so the
            # TensorEngine stays busy during the argmax/AllGather window =====
            for g in range(2):
                for k in range(NK):
                    nc.tensor.matmul(
                        ps_g[:, g, :], whh_sb[:, g, k, :], hT[:, k, :],
                        start=(g == 0 and k == 0), stop=False)
            for k in range(NK):
                nc.tensor.matmul(
                    ps_ghn[:], whh_sb[:, 2, k, :], hT[:, k, :],
                    start=(k == 0), stop=(k == NK - 1))

            # ===== local top-8 =====
            nc.vector.max(out=maxv[:], in_=logits[:])
            nc.vector.max_index(out=maxi[:], in_max=maxv[:], in_values=logits[:])
            nc.vector.tensor_copy(maxi_f[:], maxi[:, 0:KCAND])

            # ===== gather candidate [W|b] rows + exact rescore =====
            for j in range(KCAND):
                nc.gpsimd.indirect_dma_start(
                    out=g4[:, j, :],
                    out_offset=None,
                    in_=w_aug[:],
                    in_offset=bass.IndirectOffsetOnAxis(ap=maxi[:, j:j + 1], axis=0),
                )
            nc.vector.tensor_mul(
                g4[:], g4[:],
                h_aug[:].unsqueeze(1).to_broadcast([64, KCAND, 1032]))
            nc.vector.tensor_reduce(resc[:], g4[:], axis=AX.X, op=ALU.add)

            # ===== local argmax of rescored (lowest global idx on ties) =====
            nc.vector.tensor_reduce(rmax[:], resc[:], axis=AX.X, op=ALU.max)
            nc.vector.tensor_scalar(rmask[:], resc[:], rmax[:, 0:1], None, op0=ALU.is_equal)
            nc.vector.tensor_scalar_add(rtmp[:], maxi_f[:], coff_sb[:, 0:1])   # global idx
            nc.vector.tensor_scalar_add(rtmp[:], rtmp[:], -BIG)
            nc.vector.tensor_mul(rtmp[:], rtmp[:], rmask[:])
            nc.vector.tensor_scalar_add(rtmp[:], rtmp[:], BIG)
            nc.vector.tensor_reduce(lidx[:], rtmp[:], axis=AX.X, op=ALU.min)
            nc.vector.tensor_copy(agin_sb[:, 0:1], rmax[:])
            nc.vector.tensor_copy(agin_sb[:, 1:2], lidx[:])

            # ===== AllGather candidates =====
            w1 = nc.sync.dma_start(ag1_in[db][:], agin_sb[:])
            cc1 = nc.gpsimd.collective_compute(
                "AllGather", ALU.bypass,
                replica_groups=[list(range(NCORES))],
                ins=[ag1_in[db][:]], outs=[ag1_out[db][:]],
            )
            add_dep_helper(cc1.ins, w1.ins, True, "ag1 after input write")
            if prev_gg_read[db] is not None:
                add_dep_helper(cc1.ins, prev_gg_read[db].ins, True, "ag1 WAR")
            r1 = nc.sync.dma_start(
                gg[:],
                bass.AP(ag1_out[db], 0, [[2, 64], [128, 8], [1, 2]]),
            )
            add_dep_helper(r1.ins, cc1.ins, True, "gg read after ag1")
            prev_gg_read[db] = r1

            # ===== global argmax combine =====
            nc.vector.tensor_reduce(gmax[:], gg[:, :, 0], axis=AX.X, op=ALU.max)
            nc.vector.tensor_scalar(gmask[:], gg[:, :, 0], gmax[:, 0:1], None, op0=ALU.is_equal)
            nc.vector.tensor_scalar_add(gtmp[:], gg[:, :, 1], -BIG)
            nc.vector.tensor_mul(gtmp[:], gtmp[:], gmask[:])
            nc.vector.tensor_scalar_add(gtmp[:], gtmp[:], BIG)
            nc.vector.tensor_reduce(tokf[:], gtmp[:], axis=AX.X, op=ALU.min)
            nc.vector.tensor_copy(toku[:], tokf[:])
            nc.vector.tensor_copy(toks[:, t:t + 1], tokf[:])

            # ===== embedding gather + transpose =====
            nc.gpsimd.indirect_dma_start(
                out=emb_sb[:],
                out_offset=None,
                in_=wemb[:],
                in_offset=bass.IndirectOffsetOnAxis(ap=toku[:, 0:1], axis=0),
            )
            for k in range(NK):
                nc.tensor.transpose(ps_e[:, k * 64:(k + 1) * 64],
                                    emb_sb[:, k * 128:(k + 1) * 128], ident64[:])
                nc.scalar.copy(embT[:, k, :], ps_e[:, k * 64:(k + 1) * 64])

            # ===== GRU emb-side matmuls (gh side was issued just after the
            # screen; these join the same psum accumulation groups) =====
            for g in range(2):
                for k in range(NK):
                    nc.tensor.matmul(
                        ps_g[:, g, :], we_sb[:, g, k, :], embT[:, k, :],
                        start=False, stop=(g == 1 and k == NK - 1))
            for k in range(NK):
                nc.tensor.matmul(
                    ps_gin[:], we_sb[:, 2, k, :], embT[:, k, :],
                    start=(k == 0), stop=(k == NK - 1))

            # ===== gates =====
            # r = sigmoid(gi_r + gh_r + c_r)  via exp/recip
            nc.vector.tensor_add(gt1[:], ps_g[:, 0, :], cT_sb[:, 0, :])
            nc.scalar.activation(gt2[:], gt1[:], AF.Exp, scale=-1.0)
            nc.vector.tensor_scalar_add(gt2[:], gt2[:], 1.0)
            nc.vector.reciprocal(r_sb[:], gt2[:])
            # z
            nc.vector.tensor_add(gt1[:], ps_g[:, 1, :], cT_sb[:, 1, :])
            nc.scalar.activation(gt2[:], gt1[:], AF.Exp, scale=-1.0)
            nc.vector.tensor_scalar_add(gt2[:], gt2[:], 1.0)
            nc.vector.reciprocal(z_sb[:], gt2[:])
            # n = tanh(gi_n + c_n + r * (gh_n + bhh_n))
            nc.vector.tensor_scalar_add(gt1[:], ps_ghn[:], bhhn_sb[:, 0:1])
            nc.vector.tensor_mul(gt1[:], gt1[:], r_sb[:])
            nc.vector.tensor_add(gt1[:], gt1[:], ps_gin[:])
            nc.vector.tensor_add(gt1[:], gt1[:], cT_sb[:, 2, :])
            nc.scalar.activation(n_sb[:], gt1[:], AF.Tanh)
            # h_new = n + z * (h_own - n)
            nc.vector.tensor_sub(gt1[:], h_own[:], n_sb[:])
            nc.vector.tensor_mul(gt1[:], gt1[:], z_sb[:])
            nc.vector.tensor_add(hnew[:], gt1[:], n_sb[:])
            nc.vector.tensor_copy(h_own[:], hnew[:])

            # ===== AllGather h chunks =====
            w2 = nc.sync.dma_start(ag2_in[db][:], hnew[:])
            cc2 = nc.gpsimd.collective_compute(
                "AllGather", ALU.bypass,
                replica_groups=[list(range(NCORES))],
                ins=[ag2_in[db][:]], outs=[ag2_out[db][:]],
            )
            add_dep_helper(cc2.ins, w2.ins, True, "ag2 after input write")
            if prev_hT_read[db] is not None:
                add_dep_helper(cc2.ins, prev_hT_read[db].ins, True, "ag2 WAR")
            if t < steps - 1:
                r2 = nc.sync.dma_start(
                    hT[:],
                    bass.AP(ag2_out[db], 0, [[64, 128], [8192, 8], [1, 64]]),
                )
                add_dep_helper(r2.ins, cc2.ins, True, "hT read after ag2")
                prev_hT_read[db] = r2
                nc.vector.tensor_copy(hT_bf[:], hT[:])
                # rebuild h_aug (batch-major h) via PE transposes
                for k in range(NK):
                    ps_h = ps_h0 if k < 4 else ps_h1
                    kk = k % 4
                    nc.tensor.transpose(ps_h[:, kk * 128:(kk + 1) * 128],
                                        hT[:, k, :], ident128[:])
                    nc.scalar.copy(h_aug[:, k * 128:(k + 1) * 128],
                                   ps_h[:, kk * 128:(kk + 1) * 128])

        nc.sync.dma_start(out[:], toks[:])

    nc.compile()
    return nc


def prep_inputs(M, questions, word_embedding, W_out, b_out, W_ih, W_hh, b_ih, b_hh):
    """Host-side shard prep. All args np.float32 arrays."""
    f32 = np.float32
    M = np.asarray(M, f32); questions = np.asarray(questions, f32)
    word_embedding = np.ascontiguousarray(np.asarray(word_embedding, f32))
    W_out = np.asarray(W_out, f32); b_out = np.asarray(b_out, f32)
    W_ih = np.asarray(W_ih, f32); W_hh = np.asarray(W_hh, f32)
    b_ih = np.asarray(b_ih, f32); b_hh = np.asarray(b_hh, f32)

    W_pad = np.zeros((VPAD, H), f32)
    W_pad[:V] = W_out
    b_pad = np.full((VPAD,), PAD_BIAS, f32)
    b_pad[:V] = b_out

    h0 = M[:, 0, :]                      # [64, 1024]
    q = questions[:, 0, :]               # [64, 1024]
    qW = (q.astype(np.float64) @ W_ih[:, 1024:].astype(np.float64).T).astype(f32)  # [64, 3072]

    hT0 = np.ascontiguousarray(h0.T)     # [1024, 64]
    hT0_in = hT0.reshape(NK, 128, 64).transpose(1, 0, 2)  # [128, NK, 64]
    haug0 = np.zeros((64, 1032), f32)
    haug0[:, :1024] = h0
    haug0[:, 1024] = 1.0

    in_maps = []
    for c in range(NCORES):
        rows = slice(c * VSH, (c + 1) * VSH)
        Wc = W_pad[rows]                                  # [6400, 1024]
        # wt_bf [128, NK, VSH]: [p, k, v] = Wc[v, k*128+p]
        wt = Wc.T.reshape(NK, 128, VSH)                   # [k, p, v] = Wc[v, k*128+p]
        wt_bf = np.ascontiguousarray(wt.transpose(1, 0, 2)).astype(ml_dtypes.bfloat16)
        bias_bf = b_pad[rows].reshape(1, VSH).astype(ml_dtypes.bfloat16)
        w_aug = np.zeros((VSH, 1032), f32)
        w_aug[:, :1024] = Wc
        w_aug[:, 1024] = b_pad[rows]

        gr = slice(c * 128, (c + 1) * 128)
        # We rows for gates r/z/n: W_ih[g*1024 + gr, :1024]
        we = np.stack([W_ih[g * 1024 + c * 128: g * 1024 + (c + 1) * 128, :1024] for g in range(3)])   # [3, 128m, 1024]
        # we_lhsT [128p, 3, NK, 128m] = we[g, m, k*128+p]
        we_lhsT = np.ascontiguousarray(we.reshape(3, 128, NK, 128).transpose(3, 0, 2, 1))
        whh = np.stack([W_hh[g * 1024 + c * 128: g * 1024 + (c + 1) * 128, :] for g in range(3)])
        whh_lhsT = np.ascontiguousarray(whh.reshape(3, 128, NK, 128).transpose(3, 0, 2, 1))

        # cT [128p, 3, 64b]
        cT = np.zeros((128, 3, 64), f32)
        for g in range(3):
            const = qW[:, g * 1024 + c * 128: g * 1024 + (c + 1) * 128] + b_ih[g * 1024 + gr.start: g * 1024 + gr.stop]
            if g < 2:
                const = const + b_hh[g * 1024 + gr.start: g * 1024 + gr.stop]
            cT[:, g, :] = const.T
        bhh_n = b_hh[2048 + gr.start: 2048 + gr.stop].reshape(128, 1)

        h0_own = np.ascontiguousarray(h0[:, gr].T)        # [128, 64]
        coff = np.full((64, 1), c * VSH, f32)

        in_maps.append({
            "wt_bf": wt_bf,
            "bias_bf": bias_bf,
            "w_aug": w_aug,
            "wemb": word_embedding,
            "we_lhsT": we_lhsT,
            "whh_lhsT": whh_lhsT,
            "cT_in": cT,
            "bhh_n_in": bhh_n,
            "h0_own_in": h0_own,
            "hT0_in": np.ascontiguousarray(hT0_in),
            "haug0_in": haug0,
            "coff_in": coff,
        })
    return in_maps


class Runner:
    """Compile once; upload inputs and execute separately.

    Mirrors concourse.bass2jax.run_bass_via_pjrt's multi-core path, but
    keeps inputs device-resident so repeated execs measure device time
    rather than host->device transfer of ~250MB/core.
    """

    def __init__(self, nc, n_cores=NCORES):
        import jax
        from jax.experimental.shard_map import shard_map
        from jax.sharding import Mesh, PartitionSpec, NamedSharding
        from concourse import bass2jax as b2j
        from concourse import mybir

        b2j.install_neuronx_cc_hook()
        self.jax = jax
        self.nc = nc
        self.n_cores = n_cores
        partition_name = (
            nc.partition_id_tensor.name if nc.partition_id_tensor else None
        )
        in_names, out_names, out_avals, zero_outs = [], [], [], []
        for alloc in nc.m.functions[0].allocations:
            if not isinstance(alloc, mybir.MemoryLocationSet):
                continue
            name = alloc.memorylocations[0].name
            if alloc.kind == "ExternalInput":
                if name != partition_name:
                    in_names.append(name)
            elif alloc.kind == "ExternalOutput":
                shape = tuple(alloc.tensor_shape)
                dtype = mybir.dt.np(alloc.dtype)
                out_names.append(name)
                out_avals.append(jax.core.ShapedArray(shape, dtype))
                zero_outs.append(np.zeros(shape, dtype))
        n_params = len(in_names)
        n_outs = len(out_avals)
        all_in_names = list(in_names) + list(out_names)
        if partition_name is not None:
            all_in_names.append(partition_name)

        def _body(*args):
            operands = list(args)
            if partition_name is not None:
                operands.append(b2j.partition_id_tensor())
            outs = b2j._bass_exec_p.bind(
                *operands,
                out_avals=tuple(out_avals),
                in_names=tuple(all_in_names),
                out_names=tuple(out_names),
                lowering_input_output_aliases=(),
                sim_require_finite=True,
                sim_require_nnan=True,
                nc=nc,
            )
            return tuple(outs)

        devices = jax.devices()[:n_cores]
        assert len(devices) == n_cores, len(jax.devices())
        mesh = Mesh(np.asarray(devices), ("core",))
        in_specs = (PartitionSpec("core"),) * (n_params + n_outs)
        out_specs = (PartitionSpec("core"),) * n_outs
        self.sharded = jax.jit(
            shard_map(_body, mesh=mesh, in_specs=in_specs,
                      out_specs=out_specs, check_rep=False),
            donate_argnums=tuple(range(n_params, n_params + n_outs)),
            keep_unused=True,
        )
        self.sharding = NamedSharding(mesh, PartitionSpec("core"))
        self.in_names = in_names
        self.n_params = n_params
        self.out_names = out_names
        self.out_avals = out_avals
        self.zero_outs = zero_outs
        self.dev_in = None

    def upload(self, in_maps):
        concat = [
            np.concatenate(
                [np.asarray(m[name]) for m in in_maps], axis=0
            )
            for name in self.in_names
        ]
        self.dev_in = [self.jax.device_put(a, self.sharding) for a in concat]
        self.jax.block_until_ready(self.dev_in)

    def _zeros(self):
        return [
            self.jax.device_put(
                np.zeros((self.n_cores * z.shape[0], *z.shape[1:]), z.dtype),
                self.sharding,
            )
            for z in self.zero_outs
        ]

    def exec_async(self):
        return self.sharded(*self.dev_in, *self._zeros())

    def run(self):
        outs = self.exec_async()
        self.jax.block_until_ready(outs)
        return {
            name: np.asarray(outs[i]).reshape(
                self.n_cores, *self.out_avals[i].shape
            )
            for i, name in enumerate(self.out_names)
        }


_CACHE = {}


def get_runner():
    if "r" not in _CACHE:
        _CACHE["r"] = Runner(build(T))
    return _CACHE["r"]


def kernel(**inputs):
    r = get_runner()
    r.upload(prep_inputs(**inputs))
    out = r.run()["out"]
    return np.asarray(out[0], dtype=np.int32)



# revision 20
# speedup vs baseline: 6.0023x; 6.0023x over previous
"""Trainium2 Bass kernel for greedy GRU decode (AnswerModule).

B=64, H=1024, V=50257 (padded 51200), T=20 steps, 8 NeuronCores.

Strategy (tensor-parallel over vocab):
 - W_out/b_out/word-emb-gather sharded over vocab (6400 rows/core).
 - Screen: bf16 matmul h @ W_out_shard.T (+bias via K=1 matmul row) -> fp32 psum.
 - top-8 via max8/max_index; top-4 rescored exactly in fp32 via indirect-DMA
   gather of [W|b] rows + tensor_tensor_reduce dots.
 - AllGather (val,idx) -> global argmax with lowest-index tie-break.
 - Embedding gather from a replicated table; GRU sharded over H (128 rows/core)
   in fp32; AllGather h chunks.
"""
import sys
import numpy as np

sys.path.insert(0, "/opt/trn_rl_repo")
sys.path.insert(0, "/root/.axon_site")

import ml_dtypes

B = 64
H = 1024
V = 50257
VPAD = 51200
VSH = VPAD // 8          # 6400
T = 20
NCORES = 8
NK = H // 128            # 8 contraction chunks
NV = VSH // 512          # 12.5 -> handle as 12 full + 1 half? use 400-col tiles instead
# use vtile size 512 with 12 full tiles + 1 tile of 256: 12*512+256 = 6400
VT_SIZES = [512] * 12 + [256]
KCAND = 4
BIG = float(1 << 24)
PAD_BIAS = -10000.0


def build(steps=T, screen=True, rescore=True, collectives=True, embgather=True):
    import concourse.bass as bass
    import concourse.bacc as bacc
    import concourse.mybir as mybir
    from concourse import tile
    from concourse.tile_rust import add_dep_helper
    from concourse.masks import make_identity

    F32 = mybir.dt.float32
    BF16 = mybir.dt.bfloat16
    U32 = mybir.dt.uint32
    I32 = mybir.dt.int32
    AF = mybir.ActivationFunctionType
    ALU = mybir.AluOpType
    AX = mybir.AxisListType

    nc = bacc.Bacc("TRN2", target_bir_lowering=False, debug=False, num_devices=NCORES)

    # ---- external inputs (per-core shards prepared on host) ----
    wt_bf = nc.dram_tensor("wt_bf", [128, NK, VSH], BF16, kind="ExternalInput")
    bias_bf = nc.dram_tensor("bias_bf", [1, VSH], BF16, kind="ExternalInput")
    w_aug = nc.dram_tensor("w_aug", [VSH, 1032], F32, kind="ExternalInput")
    wemb = nc.dram_tensor("wemb", [V, 1024], F32, kind="ExternalInput")
    we_lhsT = nc.dram_tensor("we_lhsT", [128, 3, NK, 128], F32, kind="ExternalInput")
    whh_lhsT = nc.dram_tensor("whh_lhsT", [128, 3, NK, 128], F32, kind="ExternalInput")
    cT_in = nc.dram_tensor("cT_in", [128, 3, 64], F32, kind="ExternalInput")
    bhh_n_in = nc.dram_tensor("bhh_n_in", [128, 1], F32, kind="ExternalInput")
    h0_own_in = nc.dram_tensor("h0_own_in", [128, 64], F32, kind="ExternalInput")
    hT0_in = nc.dram_tensor("hT0_in", [128, NK, 64], F32, kind="ExternalInput")
    haug0_in = nc.dram_tensor("haug0_in", [64, 1032], F32, kind="ExternalInput")
    coff_in = nc.dram_tensor("coff_in", [64, 1], F32, kind="ExternalInput")

    out = nc.dram_tensor("out", [64, steps], I32, kind="ExternalOutput")

    # ---- collective DRAM buffers (double buffered) ----
    ag1_in = [nc.dram_tensor(f"ag1_in{i}", [64, 2], F32) for i in range(2)]
    ag1_out = [nc.dram_tensor(f"ag1_out{i}", [8, 64, 2], F32, addr_space="Shared") for i in range(2)]
    ag2_in = [nc.dram_tensor(f"ag2_in{i}", [128, 64], F32) for i in range(2)]
    ag2_out = [nc.dram_tensor(f"ag2_out{i}", [8, 128, 64], F32, addr_space="Shared") for i in range(2)]

    from contextlib import ExitStack
    ctx = ExitStack()
    with ctx:
        tc = ctx.enter_context(tile.TileContext(nc))

        # ---- sbuf tensors ----
        wt_sb = nc.alloc_sbuf_tensor("wt_sb", [128, NK, VSH], BF16)
        bias_sb = nc.alloc_sbuf_tensor("bias_sb", [1, VSH], BF16)
        ones_sb = nc.alloc_sbuf_tensor("ones_sb", [1, 64], BF16)
        we_sb = nc.alloc_sbuf_tensor("we_sb", [128, 3, NK, 128], F32)
        whh_sb = nc.alloc_sbuf_tensor("whh_sb", [128, 3, NK, 128], F32)
        cT_sb = nc.alloc_sbuf_tensor("cT_sb", [128, 3, 64], F32)
        bhhn_sb = nc.alloc_sbuf_tensor("bhhn_sb", [128, 1], F32)
        coff_sb = nc.alloc_sbuf_tensor("coff_sb", [64, 1], F32)
        ident64 = nc.alloc_sbuf_tensor("ident64", [64, 64], F32)
        ident128 = nc.alloc_sbuf_tensor("ident128", [128, 128], F32)

        hT = nc.alloc_sbuf_tensor("hT", [128, NK, 64], F32)
        hT_bf = nc.alloc_sbuf_tensor("hT_bf", [128, NK, 64], BF16)
        h_aug = nc.alloc_sbuf_tensor("h_aug", [64, 1032], F32)
        h_own = nc.alloc_sbuf_tensor("h_own", [128, 64], F32)
        hnew = nc.alloc_sbuf_tensor("hnew", [128, 64], F32)
        embT = nc.alloc_sbuf_tensor("embT", [128, NK, 64], F32)
        emb_sb = nc.alloc_sbuf_tensor("emb_sb", [64, 1024], F32)

        logits = nc.alloc_sbuf_tensor("logits", [64, VSH], F32)
        maxv = nc.alloc_sbuf_tensor("maxv", [64, 8], F32)
        maxi = nc.alloc_sbuf_tensor("maxi", [64, 8], U32)
        maxi_f = nc.alloc_sbuf_tensor("maxi_f", [64, KCAND], F32)
        g4 = nc.alloc_sbuf_tensor("g4", [64, KCAND, 1032], F32)
        resc = nc.alloc_sbuf_tensor("resc", [64, KCAND], F32)

        rmax = nc.alloc_sbuf_tensor("rmax", [64, 1], F32)
        rtmp = nc.alloc_sbuf_tensor("rtmp", [64, KCAND], F32)
        rmask = nc.alloc_sbuf_tensor("rmask", [64, KCAND], F32)
        lidx = nc.alloc_sbuf_tensor("lidx", [64, 1], F32)
        agin_sb = nc.alloc_sbuf_tensor("agin_sb", [64, 2], F32)
        gg = nc.alloc_sbuf_tensor("gg", [64, 8, 2], F32)
        gmax = nc.alloc_sbuf_tensor("gmax", [64, 1], F32)
        gmask = nc.alloc_sbuf_tensor("gmask", [64, 8], F32)
        gtmp = nc.alloc_sbuf_tensor("gtmp", [64, 8], F32)
        tokf = nc.alloc_sbuf_tensor("tokf", [64, 1], F32)
        toku = nc.alloc_sbuf_tensor("toku", [64, 1], U32)
        toks = nc.alloc_sbuf_tensor("toks", [64, steps], I32)

        r_sb = nc.alloc_sbuf_tensor("r_sb", [128, 64], F32)
        z_sb = nc.alloc_sbuf_tensor("z_sb", [128, 64], F32)
        n_sb = nc.alloc_sbuf_tensor("n_sb", [128, 64], F32)
        gt1 = nc.alloc_sbuf_tensor("gt1", [128, 64], F32)
        gt2 = nc.alloc_sbuf_tensor("gt2", [128, 64], F32)

        # ---- psum ----
        ps_scr = [ctx.enter_context(nc.psum_tensor(f"ps_scr{i}", [64, 512], F32)) for i in range(2)]
        ps_g = ctx.enter_context(nc.psum_tensor("ps_g", [128, 2, 64], F32))
        ps_ghn = ctx.enter_context(nc.psum_tensor("ps_ghn", [128, 64], F32))
        ps_gin = ctx.enter_context(nc.psum_tensor("ps_gin", [128, 64], F32))
        ps_e = ctx.enter_context(nc.psum_tensor("ps_e", [128, 512], F32))
        ps_h0 = ctx.enter_context(nc.psum_tensor("ps_h0", [64, 512], F32))
        ps_h1 = ctx.enter_context(nc.psum_tensor("ps_h1", [64, 512], F32))

        # ---- preamble: load everything ----
        nc.sync.dma_start(wt_sb[:], wt_bf[:])
        nc.sync.dma_start(bias_sb[:], bias_bf[:])
        nc.sync.dma_start(we_sb[:], we_lhsT[:])
        nc.sync.dma_start(whh_sb[:], whh_lhsT[:])
        nc.sync.dma_start(cT_sb[:], cT_in[:])
        nc.sync.dma_start(bhhn_sb[:], bhh_n_in[:])
        nc.sync.dma_start(coff_sb[:], coff_in[:])
        nc.sync.dma_start(h_own[:], h0_own_in[:])
        nc.sync.dma_start(hT[:], hT0_in[:])
        nc.sync.dma_start(h_aug[:], haug0_in[:])
        nc.vector.tensor_copy(hT_bf[:], hT[:])
        nc.vector.memset(ones_sb[:], 1.0)
        make_identity(nc, ident64[:])
        make_identity(nc, ident128[:])

        prev_gg_read = [None, None]   # for WAR dep two steps back (ag1)
        prev_hT_read = [None, None]   # (ag2)

        for t in range(steps):
            db = t % 2

            # ===== screen matmuls (bf16) + bias row =====
            if screen:
                voff = 0
                for vt, vsz in enumerate(VT_SIZES):
                    ps = ps_scr[vt % 2]
                    for k in range(NK):
                        nc.tensor.matmul(
                            ps[:, 0:vsz],
                            hT_bf[:, k, :],
                            wt_sb[:, k, voff:voff + vsz],
                            start=(k == 0), stop=False)
                    nc.tensor.matmul(
                        ps[:, 0:vsz],
                        ones_sb[:],
                        bias_sb[:, voff:voff + vsz],
                        start=False, stop=True)
                    nc.scalar.copy(logits[:, voff:voff + vsz], ps[:, 0:vsz])
                    voff += vsz

            # ===== GRU h-side matmuls (only need hT) — emitted early so the
            # TensorEngine stays busy during the argmax/AllGather window =====
            for g in range(2):
                for k in range(NK):
                    nc.tensor.matmul(
                        ps_g[:, g, :], whh_sb[:, g, k, :], hT[:, k, :],
                        start=(g == 0 and k == 0), stop=False)
            for k in range(NK):
                nc.tensor.matmul(
                    ps_ghn[:], whh_sb[:, 2, k, :], hT[:, k, :],
                    start=(k == 0), stop=(k == NK - 1))

            # ===== local top-8 =====
            if screen:
                nc.vector.max(out=maxv[:], in_=logits[:])
                nc.vector.max_index(out=maxi[:], in_max=maxv[:], in_values=logits[:])
            else:
                nc.vector.memset(maxv[:], 0.0)
                nc.vector.memset(maxi[:], 0)
            nc.vector.tensor_copy(maxi_f[:], maxi[:, 0:KCAND])

            # ===== gather candidate [W|b] rows + exact rescore =====
            if rescore:
                for j in range(KCAND):
                    nc.gpsimd.indirect_dma_start(
                        out=g4[:, j, :],
                        out_offset=None,
                        in_=w_aug[:],
                        in_offset=bass.IndirectOffsetOnAxis(ap=maxi[:, j:j + 1], axis=0),
                    )
                nc.vector.tensor_mul(
                    g4[:], g4[:],
                    h_aug[:].unsqueeze(1).to_broadcast([64, KCAND, 1032]))
                nc.vector.tensor_reduce(resc[:], g4[:], axis=AX.X, op=ALU.add)
            else:
                nc.vector.tensor_copy(resc[:], maxv[:, 0:KCAND])

            # ===== local argmax of rescored (lowest global idx on ties) =====
            nc.vector.tensor_reduce(rmax[:], resc[:], axis=AX.X, op=ALU.max)
            nc.vector.tensor_scalar(rmask[:], resc[:], rmax[:, 0:1], None, op0=ALU.is_equal)
            nc.vector.tensor_scalar_add(rtmp[:], maxi_f[:], coff_sb[:, 0:1])   # global idx
            nc.vector.tensor_scalar_add(rtmp[:], rtmp[:], -BIG)
            nc.vector.tensor_mul(rtmp[:], rtmp[:], rmask[:])
            nc.vector.tensor_scalar_add(rtmp[:], rtmp[:], BIG)
            nc.vector.tensor_reduce(lidx[:], rtmp[:], axis=AX.X, op=ALU.min)
            nc.vector.tensor_copy(agin_sb[:, 0:1], rmax[:])
            nc.vector.tensor_copy(agin_sb[:, 1:2], lidx[:])

            # ===== AllGather candidates =====
            w1 = nc.sync.dma_start(ag1_in[db][:], agin_sb[:])
            if collectives:
                cc1 = nc.gpsimd.collective_compute(
                    "AllGather", ALU.bypass,
                    replica_groups=[list(range(NCORES))],
                    ins=[ag1_in[db][:]], outs=[ag1_out[db][:]],
                )
            else:
                cc1 = nc.sync.dma_start(ag1_out[db][0], ag1_in[db][:])
            add_dep_helper(cc1.ins, w1.ins, True, "ag1 after input write")
            if prev_gg_read[db] is not None:
                add_dep_helper(cc1.ins, prev_gg_read[db].ins, True, "ag1 WAR")
            r1 = nc.sync.dma_start(
                gg[:],
                bass.AP(ag1_out[db], 0, [[2, 64], [128, 8], [1, 2]]),
            )
            add_dep_helper(r1.ins, cc1.ins, True, "gg read after ag1")
            prev_gg_read[db] = r1

            # ===== global argmax combine =====
            nc.vector.tensor_reduce(gmax[:], gg[:, :, 0], axis=AX.X, op=ALU.max)
            nc.vector.tensor_scalar(gmask[:], gg[:, :, 0], gmax[:, 0:1], None, op0=ALU.is_equal)
            nc.vector.tensor_scalar_add(gtmp[:], gg[:, :, 1], -BIG)
            nc.vector.tensor_mul(gtmp[:], gtmp[:], gmask[:])
            nc.vector.tensor_scalar_add(gtmp[:], gtmp[:], BIG)
            nc.vector.tensor_reduce(tokf[:], gtmp[:], axis=AX.X, op=ALU.min)
            # clamp to V-1 so the emb gather can't go OOB even with garbage
            # inputs (timing variants); identity for any valid token id
            nc.vector.tensor_scalar(tokf[:], tokf[:], float(V - 1), None, op0=ALU.min)
            nc.vector.tensor_copy(toku[:], tokf[:])
            nc.vector.tensor_copy(toks[:, t:t + 1], tokf[:])

            # ===== embedding gather + transpose =====
            if embgather:
                nc.gpsimd.indirect_dma_start(
                    out=emb_sb[:],
                    out_offset=None,
                    in_=wemb[:],
                    in_offset=bass.IndirectOffsetOnAxis(ap=toku[:, 0:1], axis=0),
                )
            else:
                nc.sync.dma_start(emb_sb[:], wemb[0:64, :])
            for k in range(NK):
                nc.tensor.transpose(ps_e[:, k * 64:(k + 1) * 64],
                                    emb_sb[:, k * 128:(k + 1) * 128], ident64[:])
                nc.scalar.copy(embT[:, k, :], ps_e[:, k * 64:(k + 1) * 64])

            # ===== GRU emb-side matmuls (gh side was issued just after the
            # screen; these join the same psum accumulation groups) =====
            for g in range(2):
                for k in range(NK):
                    nc.tensor.matmul(
                        ps_g[:, g, :], we_sb[:, g, k, :], embT[:, k, :],
                        start=False, stop=(g == 1 and k == NK - 1))
            for k in range(NK):
                nc.tensor.matmul(
                    ps_gin[:], we_sb[:, 2, k, :], embT[:, k, :],
                    start=(k == 0), stop=(k == NK - 1))

            # ===== gates =====
            # r = sigmoid(gi_r + gh_r + c_r)  via exp/recip
            nc.vector.tensor_add(gt1[:], ps_g[:, 0, :], cT_sb[:, 0, :])
            nc.scalar.activation(gt2[:], gt1[:], AF.Exp, scale=-1.0)
            nc.vector.tensor_scalar_add(gt2[:], gt2[:], 1.0)
            nc.vector.reciprocal(r_sb[:], gt2[:])
            # z
            nc.vector.tensor_add(gt1[:], ps_g[:, 1, :], cT_sb[:, 1, :])
            nc.scalar.activation(gt2[:], gt1[:], AF.Exp, scale=-1.0)
            nc.vector.tensor_scalar_add(gt2[:], gt2[:], 1.0)
            nc.vector.reciprocal(z_sb[:], gt2[:])
            # n = tanh(gi_n + c_n + r * (gh_n + bhh_n))
            nc.vector.tensor_scalar_add(gt1[:], ps_ghn[:], bhhn_sb[:, 0:1])
            nc.vector.tensor_mul(gt1[:], gt1[:], r_sb[:])
            nc.vector.tensor_add(gt1[:], gt1[:], ps_gin[:])
            nc.vector.tensor_add(gt1[:], gt1[:], cT_sb[:, 2, :])
            nc.scalar.activation(n_sb[:], gt1[:], AF.Tanh)
            # h_new = n + z * (h_own - n)
            nc.vector.tensor_sub(gt1[:], h_own[:], n_sb[:])
            nc.vector.tensor_mul(gt1[:], gt1[:], z_sb[:])
            nc.vector.tensor_add(hnew[:], gt1[:], n_sb[:])
            nc.vector.tensor_copy(h_own[:], hnew[:])

            # ===== AllGather h chunks =====
            w2 = nc.sync.dma_start(ag2_in[db][:], hnew[:])
            if collectives:
                cc2 = nc.gpsimd.collective_compute(
                    "AllGather", ALU.bypass,
                    replica_groups=[list(range(NCORES))],
                    ins=[ag2_in[db][:]], outs=[ag2_out[db][:]],
                )
            else:
                cc2 = nc.sync.dma_start(ag2_out[db][0], ag2_in[db][:])
            add_dep_helper(cc2.ins, w2.ins, True, "ag2 after input write")
            if prev_hT_read[db] is not None:
                add_dep_helper(cc2.ins, prev_hT_read[db].ins, True, "ag2 WAR")
            if t < steps - 1:
                r2 = nc.sync.dma_start(
                    hT[:],
                    bass.AP(ag2_out[db], 0, [[64, 128], [8192, 8], [1, 64]]),
                )
                add_dep_helper(r2.ins, cc2.ins, True, "hT read after ag2")
                prev_hT_read[db] = r2
                nc.vector.tensor_copy(hT_bf[:], hT[:])
                # rebuild h_aug (batch-major h) via PE transposes
                for k in range(NK):
                    ps_h = ps_h0 if k < 4 else ps_h1
                    kk = k % 4
                    nc.tensor.transpose(ps_h[:, kk * 128:(kk + 1) * 128],
                                        hT[:, k, :], ident128[:])
                    nc.scalar.copy(h_aug[:, k * 128:(k + 1) * 128],
                                   ps_h[:, kk * 128:(kk + 1) * 128])

        nc.sync.dma_start(out[:], toks[:])

    nc.compile()
    return nc


def prep_inputs(M, questions, word_embedding, W_out, b_out, W_ih, W_hh, b_ih, b_hh):
    """Host-side shard prep. All args np.float32 arrays."""
    f32 = np.float32
    M = np.asarray(M, f32); questions = np.asarray(questions, f32)
    word_embedding = np.ascontiguousarray(np.asarray(word_embedding, f32))
    W_out = np.asarray(W_out, f32); b_out = np.asarray(b_out, f32)
    W_ih = np.asarray(W_ih, f32); W_hh = np.asarray(W_hh, f32)
    b_ih = np.asarray(b_ih, f32); b_hh = np.asarray(b_hh, f32)

    W_pad = np.zeros((VPAD, H), f32)
    W_pad[:V] = W_out
    b_pad = np.full((VPAD,), PAD_BIAS, f32)
    b_pad[:V] = b_out

    h0 = M[:, 0, :]                      # [64, 1024]
    q = questions[:, 0, :]               # [64, 1024]
    qW = (q.astype(np.float64) @ W_ih[:, 1024:].astype(np.float64).T).astype(f32)  # [64, 3072]

    hT0 = np.ascontiguousarray(h0.T)     # [1024, 64]
    hT0_in = hT0.reshape(NK, 128, 64).transpose(1, 0, 2)  # [128, NK, 64]
    haug0 = np.zeros((64, 1032), f32)
    haug0[:, :1024] = h0
    haug0[:, 1024] = 1.0

    in_maps = []
    for c in range(NCORES):
        rows = slice(c * VSH, (c + 1) * VSH)
        Wc = W_pad[rows]                                  # [6400, 1024]
        # wt_bf [128, NK, VSH]: [p, k, v] = Wc[v, k*128+p]
        wt = Wc.T.reshape(NK, 128, VSH)                   # [k, p, v] = Wc[v, k*128+p]
        wt_bf = np.ascontiguousarray(wt.transpose(1, 0, 2)).astype(ml_dtypes.bfloat16)
        bias_bf = b_pad[rows].reshape(1, VSH).astype(ml_dtypes.bfloat16)
        w_aug = np.zeros((VSH, 1032), f32)
        w_aug[:, :1024] = Wc
        w_aug[:, 1024] = b_pad[rows]

        gr = slice(c * 128, (c + 1) * 128)
        # We rows for gates r/z/n: W_ih[g*1024 + gr, :1024]
        we = np.stack([W_ih[g * 1024 + c * 128: g * 1024 + (c + 1) * 128, :1024] for g in range(3)])   # [3, 128m, 1024]
        # we_lhsT [128p, 3, NK, 128m] = we[g, m, k*128+p]
        we_lhsT = np.ascontiguousarray(we.reshape(3, 128, NK, 128).transpose(3, 0, 2, 1))
        whh = np.stack([W_hh[g * 1024 + c * 128: g * 1024 + (c + 1) * 128, :] for g in range(3)])
        whh_lhsT = np.ascontiguousarray(whh.reshape(3, 128, NK, 128).transpose(3, 0, 2, 1))

        # cT [128p, 3, 64b]
        cT = np.zeros((128, 3, 64), f32)
        for g in range(3):
            const = qW[:, g * 1024 + c * 128: g * 1024 + (c + 1) * 128] + b_ih[g * 1024 + gr.start: g * 1024 + gr.stop]
            if g < 2:
                const = const + b_hh[g * 1024 + gr.start: g * 1024 + gr.stop]
            cT[:, g, :] = const.T
        bhh_n = b_hh[2048 + gr.start: 2048 + gr.stop].reshape(128, 1)

        h0_own = np.ascontiguousarray(h0[:, gr].T)        # [128, 64]
        coff = np.full((64, 1), c * VSH, f32)

        in_maps.append({
            "wt_bf": wt_bf,
            "bias_bf": bias_bf,
            "w_aug": w_aug,
            "wemb": word_embedding,
            "we_lhsT": we_lhsT,
            "whh_lhsT": whh_lhsT,
            "cT_in": cT,
            "bhh_n_in": bhh_n,
            "h0_own_in": h0_own,
            "hT0_in": np.ascontiguousarray(hT0_in),
            "haug0_in": haug0,
            "coff_in": coff,
        })
    return in_maps


def build_v2(steps=T, max_from_sbuf=False, idx_on_vector=False):
    """Optimized step body:
    - per-vtile top-8 (max/max_index) read PSUM directly, hidden under the
      screen matmuls; no [64,6400] logits buffer or its copies
    - candidate index extraction via is_equal + fused tensor_tensor_reduce
      min-tricks (values carry idx-BIG so min() breaks ties to lowest idx)
    - rescore dots fused (mult+add-reduce in one DVE op per candidate)
    - sigmoid gates via the activation table (validated vs f32 reference)
    """
    import concourse.bass as bass
    import concourse.bacc as bacc
    import concourse.mybir as mybir
    from concourse import tile
    from concourse.tile_rust import add_dep_helper
    from concourse.masks import make_identity

    F32 = mybir.dt.float32
    BF16 = mybir.dt.bfloat16
    U32 = mybir.dt.uint32
    I32 = mybir.dt.int32
    AF = mybir.ActivationFunctionType
    ALU = mybir.AluOpType
    AX = mybir.AxisListType

    NT = len(VT_SIZES)          # 13 vtiles
    NC8 = NT * 8                # 104 candidate slots

    nc = bacc.Bacc("TRN2", target_bir_lowering=False, debug=False, num_devices=NCORES)

    wt_bf = nc.dram_tensor("wt_bf", [128, NK, VSH], BF16, kind="ExternalInput")
    bias_bf = nc.dram_tensor("bias_bf", [1, VSH], BF16, kind="ExternalInput")
    w_aug = nc.dram_tensor("w_aug", [VSH, 1032], F32, kind="ExternalInput")
    wemb = nc.dram_tensor("wemb", [V, 1024], F32, kind="ExternalInput")
    we_lhsT = nc.dram_tensor("we_lhsT", [128, 3, NK, 128], F32, kind="ExternalInput")
    whh_lhsT = nc.dram_tensor("whh_lhsT", [128, 3, NK, 128], F32, kind="ExternalInput")
    cT_in = nc.dram_tensor("cT_in", [128, 3, 64], F32, kind="ExternalInput")
    bhh_n_in = nc.dram_tensor("bhh_n_in", [128, 1], F32, kind="ExternalInput")
    h0_own_in = nc.dram_tensor("h0_own_in", [128, 64], F32, kind="ExternalInput")
    hT0_in = nc.dram_tensor("hT0_in", [128, NK, 64], F32, kind="ExternalInput")
    haug0_in = nc.dram_tensor("haug0_in", [64, 1032], F32, kind="ExternalInput")
    coff_in = nc.dram_tensor("coff_in", [64, 1], F32, kind="ExternalInput")

    out = nc.dram_tensor("out", [64, steps], I32, kind="ExternalOutput")

    ag1_in = [nc.dram_tensor(f"ag1_in{i}", [64, 2], F32) for i in range(2)]
    ag1_out = [nc.dram_tensor(f"ag1_out{i}", [8, 64, 2], F32, addr_space="Shared") for i in range(2)]
    ag2_in = [nc.dram_tensor(f"ag2_in{i}", [128, 64], F32) for i in range(2)]
    ag2_out = [nc.dram_tensor(f"ag2_out{i}", [8, 128, 64], F32, addr_space="Shared") for i in range(2)]

    from contextlib import ExitStack
    ctx = ExitStack()
    with ctx:
        tc = ctx.enter_context(tile.TileContext(nc))

        wt_sb = nc.alloc_sbuf_tensor("wt_sb", [128, NK, VSH], BF16)
        bias_sb = nc.alloc_sbuf_tensor("bias_sb", [1, VSH], BF16)
        ones_sb = nc.alloc_sbuf_tensor("ones_sb", [1, 64], BF16)
        we_sb = nc.alloc_sbuf_tensor("we_sb", [128, 3, NK, 128], F32)
        whh_sb = nc.alloc_sbuf_tensor("whh_sb", [128, 3, NK, 128], F32)
        cT_sb = nc.alloc_sbuf_tensor("cT_sb", [128, 3, 64], F32)
        bhhn_sb = nc.alloc_sbuf_tensor("bhhn_sb", [128, 1], F32)
        coff_sb = nc.alloc_sbuf_tensor("coff_sb", [64, 1], F32)
        ident64 = nc.alloc_sbuf_tensor("ident64", [64, 64], F32)
        ident128 = nc.alloc_sbuf_tensor("ident128", [128, 128], F32)

        hT = nc.alloc_sbuf_tensor("hT", [128, NK, 64], F32)
        hT_bf = nc.alloc_sbuf_tensor("hT_bf", [128, NK, 64], BF16)
        h_aug = nc.alloc_sbuf_tensor("h_aug", [64, 1032], F32)
        h_own = nc.alloc_sbuf_tensor("h_own", [128, 64], F32)
        hnew = nc.alloc_sbuf_tensor("hnew", [128, 64], F32)
        embT = nc.alloc_sbuf_tensor("embT", [128, NK, 64], F32)
        emb_sb = nc.alloc_sbuf_tensor("emb_sb", [64, 1024], F32)

        maxv_all = nc.alloc_sbuf_tensor("maxv_all", [64, NC8], F32)
        maxi8 = nc.alloc_sbuf_tensor("maxi8", [64, NC8], U32)
        vidxB = nc.alloc_sbuf_tensor("vidxB", [64, NC8], F32)
        voffB = nc.alloc_sbuf_tensor("voffB", [64, NT], F32)
        lg_sb = [nc.alloc_sbuf_tensor(f"lg_sb{i}", [64, 512], F32) for i in range(2)] \
            if max_from_sbuf else None
        gmax8 = nc.alloc_sbuf_tensor("gmax8", [64, 8], F32)
        maskc = nc.alloc_sbuf_tensor("maskc", [64, NC8], F32)
        ttr_scr = nc.alloc_sbuf_tensor("ttr_scr", [64, NC8], F32)
        candB = nc.alloc_sbuf_tensor("candB", [64, KCAND], F32)
        cand_f = nc.alloc_sbuf_tensor("cand_f", [64, KCAND], F32)
        cand_u = nc.alloc_sbuf_tensor("cand_u", [64, KCAND], U32)
        candGB = nc.alloc_sbuf_tensor("candGB", [64, KCAND], F32)
        g4 = nc.alloc_sbuf_tensor("g4", [64, KCAND, 1032], F32)
        g4s = nc.alloc_sbuf_tensor("g4s", [64, 1032], F32)
        resc = nc.alloc_sbuf_tensor("resc", [64, KCAND], F32)

        rmax = nc.alloc_sbuf_tensor("rmax", [64, 1], F32)
        rmask = nc.alloc_sbuf_tensor("rmask", [64, KCAND], F32)
        rtt_scr = nc.alloc_sbuf_tensor("rtt_scr", [64, KCAND], F32)
        lidxB = nc.alloc_sbuf_tensor("lidxB", [64, 1], F32)
        agin_sb = nc.alloc_sbuf_tensor("agin_sb", [64, 2], F32)
        gg = nc.alloc_sbuf_tensor("gg", [64, 8, 2], F32)
        gmax = nc.alloc_sbuf_tensor("gmax", [64, 1], F32)
        gmask = nc.alloc_sbuf_tensor("gmask", [64, 8], F32)
        gtt_scr = nc.alloc_sbuf_tensor("gtt_scr", [64, 8], F32)
        tokB = nc.alloc_sbuf_tensor("tokB", [64, 1], F32)
        tokf = nc.alloc_sbuf_tensor("tokf", [64, 1], F32)
        toku = nc.alloc_sbuf_tensor("toku", [64, 1], U32)
        toks = nc.alloc_sbuf_tensor("toks", [64, steps], I32)

        r_sb = nc.alloc_sbuf_tensor("r_sb", [128, 64], F32)
        z_sb = nc.alloc_sbuf_tensor("z_sb", [128, 64], F32)
        n_sb = nc.alloc_sbuf_tensor("n_sb", [128, 64], F32)
        gt1 = nc.alloc_sbuf_tensor("gt1", [128, 64], F32)
        gt2 = nc.alloc_sbuf_tensor("gt2", [128, 64], F32)
        gt3 = nc.alloc_sbuf_tensor("gt3", [128, 64], F32)

        ps_scr = [ctx.enter_context(nc.psum_tensor(f"ps_scr{i}", [64, 512], F32)) for i in range(2)]
        ps_g = ctx.enter_context(nc.psum_tensor("ps_g", [128, 2, 64], F32))
        ps_ghn = ctx.enter_context(nc.psum_tensor("ps_ghn", [128, 64], F32))
        ps_gin = ctx.enter_context(nc.psum_tensor("ps_gin", [128, 64], F32))
        ps_e = ctx.enter_context(nc.psum_tensor("ps_e", [128, 512], F32))
        ps_h0 = ctx.enter_context(nc.psum_tensor("ps_h0", [64, 512], F32))
        ps_h1 = ctx.enter_context(nc.psum_tensor("ps_h1", [64, 512], F32))

        # ---- preamble ----
        nc.sync.dma_start(wt_sb[:], wt_bf[:])
        nc.sync.dma_start(bias_sb[:], bias_bf[:])
        nc.sync.dma_start(we_sb[:], we_lhsT[:])
        nc.sync.dma_start(whh_sb[:], whh_lhsT[:])
        nc.sync.dma_start(cT_sb[:], cT_in[:])
        nc.sync.dma_start(bhhn_sb[:], bhh_n_in[:])
        nc.sync.dma_start(coff_sb[:], coff_in[:])
        nc.sync.dma_start(h_own[:], h0_own_in[:])
        nc.sync.dma_start(hT[:], hT0_in[:])
        nc.sync.dma_start(h_aug[:], haug0_in[:])
        nc.vector.tensor_copy(hT_bf[:], hT[:])
        nc.vector.memset(ones_sb[:], 1.0)
        voff0 = 0
        for vt, vsz in enumerate(VT_SIZES):
            nc.vector.memset(voffB[:, vt:vt + 1], float(voff0 - BIG))
            voff0 += vsz
        make_identity(nc, ident64[:])
        make_identity(nc, ident128[:])

        prev_gg_read = [None, None]
        prev_hT_read = [None, None]

        for t in range(steps):
            db = t % 2

            # ===== screen + hidden per-tile top-8 =====
            voff = 0
            for vt, vsz in enumerate(VT_SIZES):
                ps = ps_scr[vt % 2]
                for k in range(NK):
                    nc.tensor.matmul(
                        ps[:, 0:vsz],
                        hT_bf[:, k, :],
                        wt_sb[:, k, voff:voff + vsz],
                        start=(k == 0), stop=False)
                nc.tensor.matmul(
                    ps[:, 0:vsz],
                    ones_sb[:],
                    bias_sb[:, voff:voff + vsz],
                    start=False, stop=True)
                sl = slice(vt * 8, vt * 8 + 8)
                if max_from_sbuf:
                    lg = lg_sb[vt % 2]
                    nc.scalar.copy(lg[:, 0:vsz], ps[:, 0:vsz])
                    src = lg[:, 0:vsz]
                else:
                    src = ps[:, 0:vsz]
                nc.vector.max(out=maxv_all[:, sl], in_=src)
                nc.vector.max_index(out=maxi8[:, sl], in_max=maxv_all[:, sl],
                                    in_values=src)
                if idx_on_vector:
                    nc.vector.tensor_copy(vidxB[:, sl], maxi8[:, sl])
                    nc.vector.tensor_scalar_add(vidxB[:, sl], vidxB[:, sl],
                                                voffB[:, vt:vt + 1])
                else:
                    # u32 idx -> f32 with +voff-BIG, on the (idle) scalar engine
                    nc.scalar.activation(vidxB[:, sl], maxi8[:, sl], AF.Identity,
                                         bias=voffB[:, vt:vt + 1], scale=1.0)
                voff += vsz

            # ===== GRU h-side matmuls keep the PE busy during argmax =====
            for g in range(2):
                for k in range(NK):
                    nc.tensor.matmul(
                        ps_g[:, g, :], whh_sb[:, g, k, :], hT[:, k, :],
                        start=(g == 0 and k == 0), stop=False)
            for k in range(NK):
                nc.tensor.matmul(
                    ps_ghn[:], whh_sb[:, 2, k, :], hT[:, k, :],
                    start=(k == 0), stop=(k == NK - 1))

            # ===== global top-4 of the 104 per-tile candidates =====
            # (values carry idx-BIG via vidxB so min() breaks ties to the
            # lowest index; non-matching slots contribute 0 > any match)
            nc.vector.max(out=gmax8[:], in_=maxv_all[:])
            for j in range(KCAND):
                nc.vector.tensor_scalar(maskc[:], maxv_all[:], gmax8[:, j:j + 1],
                                        None, op0=ALU.is_equal)
                nc.vector.tensor_mul(ttr_scr[:], maskc[:], vidxB[:])
                nc.vector.tensor_reduce(candB[:, j:j + 1], ttr_scr[:],
                                        axis=AX.X, op=ALU.min)
                nc.vector.tensor_scalar_add(cand_f[:, j:j + 1], candB[:, j:j + 1], BIG)
                nc.vector.tensor_copy(cand_u[:, j:j + 1], cand_f[:, j:j + 1])
                nc.gpsimd.indirect_dma_start(
                    out=g4[:, j, :],
                    out_offset=None,
                    in_=w_aug[:],
                    in_offset=bass.IndirectOffsetOnAxis(ap=cand_u[:, j:j + 1], axis=0),
                )

            # ===== exact rescore: mul on VectorE, sum on ScalarE =====
            for j in range(KCAND):
                nc.vector.tensor_mul(g4[:, j, :], g4[:, j, :], h_aug[:])
                nc.scalar.activation(g4s[:], g4[:, j, :], AF.Identity,
                                     accum_out=resc[:, j:j + 1])

            # ===== local argmax (lowest global idx on ties) =====
            nc.vector.tensor_scalar_add(candGB[:], candB[:], coff_sb[:, 0:1])
            nc.vector.tensor_reduce(rmax[:], resc[:], axis=AX.X, op=ALU.max)
            nc.vector.tensor_scalar(rmask[:], resc[:], rmax[:, 0:1], None, op0=ALU.is_equal)
            nc.vector.tensor_mul(rtt_scr[:], rmask[:], candGB[:])
            nc.vector.tensor_reduce(lidxB[:], rtt_scr[:], axis=AX.X, op=ALU.min)
            nc.vector.tensor_copy(agin_sb[:, 0:1], rmax[:])
            nc.vector.tensor_copy(agin_sb[:, 1:2], lidxB[:])

            # ===== AllGather candidates =====
            w1 = nc.sync.dma_start(ag1_in[db][:], agin_sb[:])
            cc1 = nc.gpsimd.collective_compute(
                "AllGather", ALU.bypass,
                replica_groups=[list(range(NCORES))],
                ins=[ag1_in[db][:]], outs=[ag1_out[db][:]],
            )
            add_dep_helper(cc1.ins, w1.ins, True, "ag1 after input write")
            if prev_gg_read[db] is not None:
                add_dep_helper(cc1.ins, prev_gg_read[db].ins, True, "ag1 WAR")
            r1 = nc.sync.dma_start(
                gg[:],
                bass.AP(ag1_out[db], 0, [[2, 64], [128, 8], [1, 2]]),
            )
            add_dep_helper(r1.ins, cc1.ins, True, "gg read after ag1")
            prev_gg_read[db] = r1

            # ===== global argmax combine =====
            nc.vector.tensor_reduce(gmax[:], gg[:, :, 0], axis=AX.X, op=ALU.max)
            nc.vector.tensor_scalar(gmask[:], gg[:, :, 0], gmax[:, 0:1], None, op0=ALU.is_equal)
            nc.vector.tensor_mul(gtt_scr[:], gmask[:], gg[:, :, 1])
            nc.vector.tensor_reduce(tokB[:], gtt_scr[:], axis=AX.X, op=ALU.min)
            nc.vector.tensor_scalar_add(tokf[:], tokB[:], BIG)
            nc.vector.tensor_scalar(tokf[:], tokf[:], float(V - 1), None, op0=ALU.min)
            nc.vector.tensor_copy(toku[:], tokf[:])
            nc.vector.tensor_copy(toks[:, t:t + 1], tokf[:])

            # ===== embedding gather + transpose =====
            nc.gpsimd.indirect_dma_start(
                out=emb_sb[:],
                out_offset=None,
                in_=wemb[:],
                in_offset=bass.IndirectOffsetOnAxis(ap=toku[:, 0:1], axis=0),
            )
            for k in range(NK):
                nc.tensor.transpose(ps_e[:, k * 64:(k + 1) * 64],
                                    emb_sb[:, k * 128:(k + 1) * 128], ident64[:])
                nc.scalar.copy(embT[:, k, :], ps_e[:, k * 64:(k + 1) * 64])

            # ===== GRU emb-side matmuls =====
            for g in range(2):
                for k in range(NK):
                    nc.tensor.matmul(
                        ps_g[:, g, :], we_sb[:, g, k, :], embT[:, k, :],
                        start=False, stop=(g == 1 and k == NK - 1))
            for k in range(NK):
                nc.tensor.matmul(
                    ps_gin[:], we_sb[:, 2, k, :], embT[:, k, :],
                    start=(k == 0), stop=(k == NK - 1))

            # ===== gates (sigmoid/tanh via activation table) =====
            nc.vector.tensor_add(gt1[:], ps_g[:, 0, :], cT_sb[:, 0, :])
            nc.scalar.activation(r_sb[:], gt1[:], AF.Sigmoid)
            nc.vector.tensor_add(gt2[:], ps_g[:, 1, :], cT_sb[:, 1, :])
            nc.scalar.activation(z_sb[:], gt2[:], AF.Sigmoid)
            nc.vector.tensor_scalar_add(gt3[:], ps_ghn[:], bhhn_sb[:, 0:1])
            nc.vector.tensor_mul(gt3[:], gt3[:], r_sb[:])
            nc.vector.tensor_add(gt3[:], gt3[:], ps_gin[:])
            nc.vector.tensor_add(gt3[:], gt3[:], cT_sb[:, 2, :])
            nc.scalar.activation(n_sb[:], gt3[:], AF.Tanh)
            nc.vector.tensor_sub(gt3[:], h_own[:], n_sb[:])
            nc.vector.tensor_mul(gt3[:], gt3[:], z_sb[:])
            nc.vector.tensor_add(hnew[:], gt3[:], n_sb[:])
            nc.vector.tensor_copy(h_own[:], hnew[:])

            # ===== AllGather h chunks =====
            w2 = nc.sync.dma_start(ag2_in[db][:], hnew[:])
            cc2 = nc.gpsimd.collective_compute(
                "AllGather", ALU.bypass,
                replica_groups=[list(range(NCORES))],
                ins=[ag2_in[db][:]], outs=[ag2_out[db][:]],
            )
            add_dep_helper(cc2.ins, w2.ins, True, "ag2 after input write")
            if prev_hT_read[db] is not None:
                add_dep_helper(cc2.ins, prev_hT_read[db].ins, True, "ag2 WAR")
            if t < steps - 1:
                r2 = nc.sync.dma_start(
                    hT[:],
                    bass.AP(ag2_out[db], 0, [[64, 128], [8192, 8], [1, 64]]),
                )
                add_dep_helper(r2.ins, cc2.ins, True, "hT read after ag2")
                prev_hT_read[db] = r2
                nc.vector.tensor_copy(hT_bf[:], hT[:])
                for k in range(NK):
                    ps_h = ps_h0 if k < 4 else ps_h1
                    kk = k % 4
                    nc.tensor.transpose(ps_h[:, kk * 128:(kk + 1) * 128],
                                        hT[:, k, :], ident128[:])
                    nc.scalar.copy(h_aug[:, k * 128:(k + 1) * 128],
                                   ps_h[:, kk * 128:(kk + 1) * 128])

        nc.sync.dma_start(out[:], toks[:])

    nc.compile()
    return nc


def build_v3(steps=T):
    """v2 plus:
    - k-outer screen in groups of 4 vtiles with per-chunk hT reads, so the
      next screen starts as soon as the first h chunk lands after ag2
    - ag1 payload transposed to [2,64] so collective reads are
      descriptor-light ([8,64] contiguous) and re-transposed on the PE
    - GRU psums consolidated into one bank to free banks for the 4-deep
      screen pipeline
    """
    import concourse.bass as bass
    import concourse.bacc as bacc
    import concourse.mybir as mybir
    from concourse import tile
    from concourse.tile_rust import add_dep_helper
    from concourse.masks import make_identity

    F32 = mybir.dt.float32
    BF16 = mybir.dt.bfloat16
    U32 = mybir.dt.uint32
    I32 = mybir.dt.int32
    AF = mybir.ActivationFunctionType
    ALU = mybir.AluOpType
    AX = mybir.AxisListType

    NT = len(VT_SIZES)
    NC8 = NT * 8
    GROUPS = [list(range(g, min(g + 4, NT))) for g in range(0, NT, 4)]
    VOFFS = []
    _vo = 0
    for vs in VT_SIZES:
        VOFFS.append(_vo)
        _vo += vs

    nc = bacc.Bacc("TRN2", target_bir_lowering=False, debug=False, num_devices=NCORES)

    wt_bf = nc.dram_tensor("wt_bf", [128, NK, VSH], BF16, kind="ExternalInput")
    bias_bf = nc.dram_tensor("bias_bf", [1, VSH], BF16, kind="ExternalInput")
    w_aug = nc.dram_tensor("w_aug", [VSH, 1032], F32, kind="ExternalInput")
    wemb = nc.dram_tensor("wemb", [V, 1024], F32, kind="ExternalInput")
    we_lhsT = nc.dram_tensor("we_lhsT", [128, 3, NK, 128], F32, kind="ExternalInput")
    whh_lhsT = nc.dram_tensor("whh_lhsT", [128, 3, NK, 128], F32, kind="ExternalInput")
    cT_in = nc.dram_tensor("cT_in", [128, 3, 64], F32, kind="ExternalInput")
    bhh_n_in = nc.dram_tensor("bhh_n_in", [128, 1], F32, kind="ExternalInput")
    h0_own_in = nc.dram_tensor("h0_own_in", [128, 64], F32, kind="ExternalInput")
    hT0_in = nc.dram_tensor("hT0_in", [128, NK, 64], F32, kind="ExternalInput")
    haug0_in = nc.dram_tensor("haug0_in", [64, 1032], F32, kind="ExternalInput")
    coff_in = nc.dram_tensor("coff_in", [64, 1], F32, kind="ExternalInput")

    out = nc.dram_tensor("out", [64, steps], I32, kind="ExternalOutput")

    ag1_in = [nc.dram_tensor(f"ag1_in{i}", [2, 64], F32) for i in range(2)]
    ag1_out = [nc.dram_tensor(f"ag1_out{i}", [8, 2, 64], F32, addr_space="Shared") for i in range(2)]
    ag2_in = [nc.dram_tensor(f"ag2_in{i}", [128, 64], F32) for i in range(2)]
    ag2_out = [nc.dram_tensor(f"ag2_out{i}", [8, 128, 64], F32, addr_space="Shared") for i in range(2)]

    from contextlib import ExitStack
    ctx = ExitStack()
    with ctx:
        tc = ctx.enter_context(tile.TileContext(nc))

        wt_sb = nc.alloc_sbuf_tensor("wt_sb", [128, NK, VSH], BF16)
        bias_sb = nc.alloc_sbuf_tensor("bias_sb", [1, VSH], BF16)
        ones_sb = nc.alloc_sbuf_tensor("ones_sb", [1, 64], BF16)
        we_sb = nc.alloc_sbuf_tensor("we_sb", [128, 3, NK, 128], F32)
        whh_sb = nc.alloc_sbuf_tensor("whh_sb", [128, 3, NK, 128], F32)
        cT_sb = nc.alloc_sbuf_tensor("cT_sb", [128, 3, 64], F32)
        bhhn_sb = nc.alloc_sbuf_tensor("bhhn_sb", [128, 1], F32)
        coff_sb = nc.alloc_sbuf_tensor("coff_sb", [64, 1], F32)
        ident64 = nc.alloc_sbuf_tensor("ident64", [64, 64], F32)
        ident128 = nc.alloc_sbuf_tensor("ident128", [128, 128], F32)

        hT = nc.alloc_sbuf_tensor("hT", [128, NK, 64], F32)
        hT_bf = nc.alloc_sbuf_tensor("hT_bf", [128, NK, 64], BF16)
        h_aug = nc.alloc_sbuf_tensor("h_aug", [64, 1032], F32)
        h_own = nc.alloc_sbuf_tensor("h_own", [128, 64], F32)
        hnew = nc.alloc_sbuf_tensor("hnew", [128, 64], F32)
        embT = nc.alloc_sbuf_tensor("embT", [128, NK, 64], F32)
        emb_sb = nc.alloc_sbuf_tensor("emb_sb", [64, 1024], F32)

        maxv_all = nc.alloc_sbuf_tensor("maxv_all", [64, NC8], F32)
        maxi8 = nc.alloc_sbuf_tensor("maxi8", [64, NC8], U32)
        vidxB = nc.alloc_sbuf_tensor("vidxB", [64, NC8], F32)
        voffB = nc.alloc_sbuf_tensor("voffB", [64, NT], F32)
        gmax8 = nc.alloc_sbuf_tensor("gmax8", [64, 8], F32)
        maskc = nc.alloc_sbuf_tensor("maskc", [64, NC8], F32)
        ttr_scr = nc.alloc_sbuf_tensor("ttr_scr", [64, NC8], F32)
        candB = nc.alloc_sbuf_tensor("candB", [64, KCAND], F32)
        cand_f = nc.alloc_sbuf_tensor("cand_f", [64, KCAND], F32)
        cand_u = nc.alloc_sbuf_tensor("cand_u", [64, KCAND], U32)
        candGB = nc.alloc_sbuf_tensor("candGB", [64, KCAND], F32)
        g4 = nc.alloc_sbuf_tensor("g4", [64, KCAND, 1032], F32)
        g4s = nc.alloc_sbuf_tensor("g4s", [64, 1032], F32)
        resc = nc.alloc_sbuf_tensor("resc", [64, KCAND], F32)

        rmax = nc.alloc_sbuf_tensor("rmax", [64, 1], F32)
        rmask = nc.alloc_sbuf_tensor("rmask", [64, KCAND], F32)
        rtt_scr = nc.alloc_sbuf_tensor("rtt_scr", [64, KCAND], F32)
        lidxB = nc.alloc_sbuf_tensor("lidxB", [64, 1], F32)
        agin_sb = nc.alloc_sbuf_tensor("agin_sb", [64, 2], F32)
        aginT_sb = nc.alloc_sbuf_tensor("aginT_sb", [2, 64], F32)
        gv8 = nc.alloc_sbuf_tensor("gv8", [8, 64], F32)
        gi8 = nc.alloc_sbuf_tensor("gi8", [8, 64], F32)
        gmaxv8 = nc.alloc_sbuf_tensor("gmaxv8", [64, 8], F32)
        gidx8 = nc.alloc_sbuf_tensor("gidx8", [64, 8], F32)
        gmax = nc.alloc_sbuf_tensor("gmax", [64, 1], F32)
        gmask = nc.alloc_sbuf_tensor("gmask", [64, 8], F32)
        gtt_scr = nc.alloc_sbuf_tensor("gtt_scr", [64, 8], F32)
        tokB = nc.alloc_sbuf_tensor("tokB", [64, 1], F32)
        tokf = nc.alloc_sbuf_tensor("tokf", [64, 1], F32)
        toku = nc.alloc_sbuf_tensor("toku", [64, 1], U32)
        toks = nc.alloc_sbuf_tensor("toks", [64, steps], I32)

        r_sb = nc.alloc_sbuf_tensor("r_sb", [128, 64], F32)
        z_sb = nc.alloc_sbuf_tensor("z_sb", [128, 64], F32)
        n_sb = nc.alloc_sbuf_tensor("n_sb", [128, 64], F32)
        gt1 = nc.alloc_sbuf_tensor("gt1", [128, 64], F32)
        gt2 = nc.alloc_sbuf_tensor("gt2", [128, 64], F32)
        gt3 = nc.alloc_sbuf_tensor("gt3", [128, 64], F32)

        ps_scr = [ctx.enter_context(nc.psum_tensor(f"ps_scr{i}", [64, 512], F32)) for i in range(4)]
        ps_gru = ctx.enter_context(nc.psum_tensor("ps_gru", [128, 4, 64], F32))
        ps_e = ctx.enter_context(nc.psum_tensor("ps_e", [128, 512], F32))
        ps_h0 = ctx.enter_context(nc.psum_tensor("ps_h0", [64, 512], F32))
        ps_h1 = ctx.enter_context(nc.psum_tensor("ps_h1", [64, 512], F32))

        # ---- preamble ----
        nc.sync.dma_start(wt_sb[:], wt_bf[:])
        nc.sync.dma_start(bias_sb[:], bias_bf[:])
        nc.sync.dma_start(we_sb[:], we_lhsT[:])
        nc.sync.dma_start(whh_sb[:], whh_lhsT[:])
        nc.sync.dma_start(cT_sb[:], cT_in[:])
        nc.sync.dma_start(bhhn_sb[:], bhh_n_in[:])
        nc.sync.dma_start(coff_sb[:], coff_in[:])
        nc.sync.dma_start(h_own[:], h0_own_in[:])
        nc.sync.dma_start(hT[:], hT0_in[:])
        nc.sync.dma_start(h_aug[:], haug0_in[:])
        nc.vector.tensor_copy(hT_bf[:], hT[:])
        nc.vector.memset(ones_sb[:], 1.0)
        for vt in range(NT):
            nc.vector.memset(voffB[:, vt:vt + 1], float(VOFFS[vt] - BIG))
        make_identity(nc, ident64[:])
        make_identity(nc, ident128[:])

        prev_gg_read = [None, None]   # list of read instrs per db
        prev_hT_read = [None, None]

        for t in range(steps):
            db = t % 2

            # ===== screen: groups of 4 vtiles, k-outer inside a group =====
            for grp in GROUPS:
                for k in range(NK):
                    for vt in grp:
                        nc.tensor.matmul(
                            ps_scr[vt % 4][:, 0:VT_SIZES[vt]],
                            hT_bf[:, k, :],
                            wt_sb[:, k, VOFFS[vt]:VOFFS[vt] + VT_SIZES[vt]],
                            start=(k == 0), stop=False)
                for vt in grp:
                    nc.tensor.matmul(
                        ps_scr[vt % 4][:, 0:VT_SIZES[vt]],
                        ones_sb[:],
                        bias_sb[:, VOFFS[vt]:VOFFS[vt] + VT_SIZES[vt]],
                        start=False, stop=True)
                for vt in grp:
                    vsz = VT_SIZES[vt]
                    sl = slice(vt * 8, vt * 8 + 8)
                    nc.vector.max(out=maxv_all[:, sl], in_=ps_scr[vt % 4][:, 0:vsz])
                    nc.vector.max_index(out=maxi8[:, sl], in_max=maxv_all[:, sl],
                                        in_values=ps_scr[vt % 4][:, 0:vsz])
                    nc.scalar.activation(vidxB[:, sl], maxi8[:, sl], AF.Identity,
                                         bias=voffB[:, vt:vt + 1], scale=1.0)

            # ===== GRU h-side matmuls keep the PE busy during argmax =====
            for g in range(2):
                for k in range(NK):
                    nc.tensor.matmul(
                        ps_gru[:, g, :], whh_sb[:, g, k, :], hT[:, k, :],
                        start=(g == 0 and k == 0), stop=False)
            for k in range(NK):
                nc.tensor.matmul(
                    ps_gru[:, 2, :], whh_sb[:, 2, k, :], hT[:, k, :],
                    start=(k == 0), stop=(k == NK - 1))

            # ===== global top-4 of the 104 per-tile candidates =====
            nc.vector.max(out=gmax8[:], in_=maxv_all[:])
            for j in range(KCAND):
                nc.vector.tensor_scalar(maskc[:], maxv_all[:], gmax8[:, j:j + 1],
                                        None, op0=ALU.is_equal)
                nc.vector.tensor_mul(ttr_scr[:], maskc[:], vidxB[:])
                nc.vector.tensor_reduce(candB[:, j:j + 1], ttr_scr[:],
                                        axis=AX.X, op=ALU.min)
                nc.vector.tensor_scalar_add(cand_f[:, j:j + 1], candB[:, j:j + 1], BIG)
                nc.vector.tensor_copy(cand_u[:, j:j + 1], cand_f[:, j:j + 1])
                nc.gpsimd.indirect_dma_start(
                    out=g4[:, j, :],
                    out_offset=None,
                    in_=w_aug[:],
                    in_offset=bass.IndirectOffsetOnAxis(ap=cand_u[:, j:j + 1], axis=0),
                )

            # ===== exact rescore: mul on VectorE, sum on ScalarE =====
            for j in range(KCAND):
                nc.vector.tensor_mul(g4[:, j, :], g4[:, j, :], h_aug[:])
                nc.scalar.activation(g4s[:], g4[:, j, :], AF.Identity,
                                     accum_out=resc[:, j:j + 1])

            # ===== local argmax (lowest global idx on ties) =====
            nc.vector.tensor_scalar_add(candGB[:], candB[:], coff_sb[:, 0:1])
            nc.vector.tensor_reduce(rmax[:], resc[:], axis=AX.X, op=ALU.max)
            nc.vector.tensor_scalar(rmask[:], resc[:], rmax[:, 0:1], None, op0=ALU.is_equal)
            nc.vector.tensor_mul(rtt_scr[:], rmask[:], candGB[:])
            nc.vector.tensor_reduce(lidxB[:], rtt_scr[:], axis=AX.X, op=ALU.min)
            nc.vector.tensor_copy(agin_sb[:, 0:1], rmax[:])
            nc.vector.tensor_copy(agin_sb[:, 1:2], lidxB[:])
            # transpose [64,2] -> [2,64] so the collective transport is
            # contiguous on both sides
            nc.tensor.transpose(ps_e[0:2, 0:64], agin_sb[:], ident64[:])
            nc.scalar.copy(aginT_sb[:], ps_e[0:2, 0:64])

            # ===== AllGather candidates =====
            w1 = nc.sync.dma_start(ag1_in[db][:], aginT_sb[:])
            cc1 = nc.gpsimd.collective_compute(
                "AllGather", ALU.bypass,
                replica_groups=[list(range(NCORES))],
                ins=[ag1_in[db][:]], outs=[ag1_out[db][:]],
            )
            add_dep_helper(cc1.ins, w1.ins, True, "ag1 after input write")
            if prev_gg_read[db] is not None:
                for pr in prev_gg_read[db]:
                    add_dep_helper(cc1.ins, pr.ins, True, "ag1 WAR")
            # vals at rows 0,2,..14; idx at rows 1,3,..15 of [16,64]
            r1v = nc.sync.dma_start(
                gv8[:], bass.AP(ag1_out[db], 0, [[128, 8], [1, 64]]))
            r1i = nc.sync.dma_start(
                gi8[:], bass.AP(ag1_out[db], 64, [[128, 8], [1, 64]]))
            add_dep_helper(r1v.ins, cc1.ins, True, "gv8 read after ag1")
            add_dep_helper(r1i.ins, cc1.ins, True, "gi8 read after ag1")
            prev_gg_read[db] = [r1v, r1i]
            nc.tensor.transpose(ps_e[0:64, 64:72], gv8[:], ident64[0:8, 0:8])
            nc.tensor.transpose(ps_e[0:64, 72:80], gi8[:], ident64[0:8, 0:8])
            nc.scalar.copy(gmaxv8[:], ps_e[0:64, 64:72])
            nc.scalar.copy(gidx8[:], ps_e[0:64, 72:80])

            # ===== global argmax combine =====
            nc.vector.tensor_reduce(gmax[:], gmaxv8[:], axis=AX.X, op=ALU.max)
            nc.vector.tensor_scalar(gmask[:], gmaxv8[:], gmax[:, 0:1], None, op0=ALU.is_equal)
            nc.vector.tensor_mul(gtt_scr[:], gmask[:], gidx8[:])
            nc.vector.tensor_reduce(tokB[:], gtt_scr[:], axis=AX.X, op=ALU.min)
            nc.vector.tensor_scalar_add(tokf[:], tokB[:], BIG)
            nc.vector.tensor_scalar(tokf[:], tokf[:], float(V - 1), None, op0=ALU.min)
            nc.vector.tensor_copy(toku[:], tokf[:])
            nc.vector.tensor_copy(toks[:, t:t + 1], tokf[:])

            # ===== embedding gather + transpose =====
            nc.gpsimd.indirect_dma_start(
                out=emb_sb[:],
                out_offset=None,
                in_=wemb[:],
                in_offset=bass.IndirectOffsetOnAxis(ap=toku[:, 0:1], axis=0),
            )
            for k in range(NK):
                nc.tensor.transpose(ps_e[:, k * 64:(k + 1) * 64],
                                    emb_sb[:, k * 128:(k + 1) * 128], ident64[:])
                nc.scalar.copy(embT[:, k, :], ps_e[:, k * 64:(k + 1) * 64])

            # ===== GRU emb-side matmuls =====
            for g in range(2):
                for k in range(NK):
                    nc.tensor.matmul(
                        ps_gru[:, g, :], we_sb[:, g, k, :], embT[:, k, :],
                        start=False, stop=(g == 1 and k == NK - 1))
            for k in range(NK):
                nc.tensor.matmul(
                    ps_gru[:, 3, :], we_sb[:, 2, k, :], embT[:, k, :],
                    start=(k == 0), stop=(k == NK - 1))

            # ===== gates =====
            nc.vector.tensor_add(gt1[:], ps_gru[:, 0, :], cT_sb[:, 0, :])
            nc.scalar.activation(r_sb[:], gt1[:], AF.Sigmoid)
            nc.vector.tensor_add(gt2[:], ps_gru[:, 1, :], cT_sb[:, 1, :])
            nc.scalar.activation(z_sb[:], gt2[:], AF.Sigmoid)
            nc.vector.tensor_scalar_add(gt3[:], ps_gru[:, 2, :], bhhn_sb[:, 0:1])
            nc.vector.tensor_mul(gt3[:], gt3[:], r_sb[:])
            nc.vector.tensor_add(gt3[:], gt3[:], ps_gru[:, 3, :])
            nc.vector.tensor_add(gt3[:], gt3[:], cT_sb[:, 2, :])
            nc.scalar.activation(n_sb[:], gt3[:], AF.Tanh)
            nc.vector.tensor_sub(gt3[:], h_own[:], n_sb[:])
            nc.vector.tensor_mul(gt3[:], gt3[:], z_sb[:])
            nc.vector.tensor_add(hnew[:], gt3[:], n_sb[:])
            nc.vector.tensor_copy(h_own[:], hnew[:])

            # ===== AllGather h chunks; per-chunk reads feed the next screen =====
            w2 = nc.sync.dma_start(ag2_in[db][:], hnew[:])
            cc2 = nc.gpsimd.collective_compute(
                "AllGather", ALU.bypass,
                replica_groups=[list(range(NCORES))],
                ins=[ag2_in[db][:]], outs=[ag2_out[db][:]],
            )
            add_dep_helper(cc2.ins, w2.ins, True, "ag2 after input write")
            if prev_hT_read[db] is not None:
                for pr in prev_hT_read[db]:
                    add_dep_helper(cc2.ins, pr.ins, True, "ag2 WAR")
            if t < steps - 1:
                reads = []
                for k in range(NK):
                    rk = nc.sync.dma_start(hT[:, k, :], ag2_out[db][k])
                    add_dep_helper(rk.ins, cc2.ins, True, "hT chunk after ag2")
                    reads.append(rk)
                    nc.vector.tensor_copy(hT_bf[:, k, :], hT[:, k, :])
                prev_hT_read[db] = reads
                for k in range(NK):
                    ps_h = ps_h0 if k < 4 else ps_h1
                    kk = k % 4
                    nc.tensor.transpose(ps_h[:, kk * 128:(kk + 1) * 128],
                                        hT[:, k, :], ident128[:])
                    nc.scalar.copy(h_aug[:, k * 128:(k + 1) * 128],
                                   ps_h[:, kk * 128:(kk + 1) * 128])

        nc.sync.dma_start(out[:], toks[:])

    nc.compile()
    return nc


class Runner:
    """Compile once; upload inputs and execute separately.

    Mirrors concourse.bass2jax.run_bass_via_pjrt's multi-core path, but
    keeps inputs device-resident so repeated execs measure device time
    rather than host->device transfer of ~250MB/core.
    """

    def __init__(self, nc, n_cores=NCORES):
        import jax
        from jax.experimental.shard_map import shard_map
        from jax.sharding import Mesh, PartitionSpec, NamedSharding
        from concourse import bass2jax as b2j
        from concourse import mybir

        b2j.install_neuronx_cc_hook()
        self.jax = jax
        self.nc = nc
        self.n_cores = n_cores
        partition_name = (
            nc.partition_id_tensor.name if nc.partition_id_tensor else None
        )
        in_names, out_names, out_avals, zero_outs = [], [], [], []
        for alloc in nc.m.functions[0].allocations:
            if not isinstance(alloc, mybir.MemoryLocationSet):
                continue
            name = alloc.memorylocations[0].name
            if alloc.kind == "ExternalInput":
                if name != partition_name:
                    in_names.append(name)
            elif alloc.kind == "ExternalOutput":
                shape = tuple(alloc.tensor_shape)
                dtype = mybir.dt.np(alloc.dtype)
                out_names.append(name)
                out_avals.append(jax.core.ShapedArray(shape, dtype))
                zero_outs.append(np.zeros(shape, dtype))
        n_params = len(in_names)
        n_outs = len(out_avals)
        all_in_names = list(in_names) + list(out_names)
        if partition_name is not None:
            all_in_names.append(partition_name)

        def _body(*args):
            operands = list(args)
            if partition_name is not None:
                operands.append(b2j.partition_id_tensor())
            outs = b2j._bass_exec_p.bind(
                *operands,
                out_avals=tuple(out_avals),
                in_names=tuple(all_in_names),
                out_names=tuple(out_names),
                lowering_input_output_aliases=(),
                sim_require_finite=True,
                sim_require_nnan=True,
                nc=nc,
            )
            return tuple(outs)

        devices = jax.devices()[:n_cores]
        assert len(devices) == n_cores, len(jax.devices())
        mesh = Mesh(np.asarray(devices), ("core",))
        in_specs = (PartitionSpec("core"),) * (n_params + n_outs)
        out_specs = (PartitionSpec("core"),) * n_outs
        self.sharded = jax.jit(
            shard_map(_body, mesh=mesh, in_specs=in_specs,
                      out_specs=out_specs, check_rep=False),
            donate_argnums=tuple(range(n_params, n_params + n_outs)),
            keep_unused=True,
        )
        self.sharding = NamedSharding(mesh, PartitionSpec("core"))
        self.in_names = in_names
        self.n_params = n_params
        self.out_names = out_names
        self.out_avals = out_avals
        self.zero_outs = zero_outs
        self.dev_in = None

    def upload(self, in_maps):
        concat = [
            np.concatenate(
                [np.asarray(m[name]) for m in in_maps], axis=0
            )
            for name in self.in_names
        ]
        self.dev_in = [self.jax.device_put(a, self.sharding) for a in concat]
        self.jax.block_until_ready(self.dev_in)

    def _zeros(self):
        return [
            self.jax.device_put(
                np.zeros((self.n_cores * z.shape[0], *z.shape[1:]), z.dtype),
                self.sharding,
            )
            for z in self.zero_outs
        ]

    def exec_async(self):
        return self.sharded(*self.dev_in, *self._zeros())

    def run(self):
        outs = self.exec_async()
        self.jax.block_until_ready(outs)
        return {
            name: np.asarray(outs[i]).reshape(
                self.n_cores, *self.out_avals[i].shape
            )
            for i, name in enumerate(self.out_names)
        }


_CACHE = {}


def get_runner():
    if "r" not in _CACHE:
        _CACHE["r"] = Runner(build_v2(T))
    return _CACHE["r"]


def kernel(**inputs):
    r = get_runner()
    r.upload(prep_inputs(**inputs))
    out = r.run()["out"]
    return np.asarray(out[0], dtype=np.int32)



# revision 24
# speedup vs baseline: 6.0141x; 1.0020x over previous
"""Trainium2 Bass kernel for greedy GRU decode (AnswerModule).

B=64, H=1024, V=50257 (padded 51200), T=20 steps, 8 NeuronCores.

Strategy (tensor-parallel over vocab):
 - W_out/b_out/word-emb-gather sharded over vocab (6400 rows/core).
 - Screen: bf16 matmul h @ W_out_shard.T (+bias via K=1 matmul row) -> fp32 psum.
 - top-8 via max8/max_index; top-4 rescored exactly in fp32 via indirect-DMA
   gather of [W|b] rows + tensor_tensor_reduce dots.
 - AllGather (val,idx) -> global argmax with lowest-index tie-break.
 - Embedding gather from a replicated table; GRU sharded over H (128 rows/core)
   in fp32; AllGather h chunks.
"""
import sys
import numpy as np

sys.path.insert(0, "/opt/trn_rl_repo")
sys.path.insert(0, "/root/.axon_site")

import ml_dtypes

B = 64
H = 1024
V = 50257
VPAD = 51200
VSH = VPAD // 8          # 6400
T = 20
NCORES = 8
NK = H // 128            # 8 contraction chunks
NV = VSH // 512          # 12.5 -> handle as 12 full + 1 half? use 400-col tiles instead
# use vtile size 512 with 12 full tiles + 1 tile of 256: 12*512+256 = 6400
VT_SIZES = [512] * 12 + [256]
KCAND = 4
BIG = float(1 << 24)
PAD_BIAS = -10000.0


def build(steps=T, screen=True, rescore=True, collectives=True, embgather=True):
    import concourse.bass as bass
    import concourse.bacc as bacc
    import concourse.mybir as mybir
    from concourse import tile
    from concourse.tile_rust import add_dep_helper
    from concourse.masks import make_identity

    F32 = mybir.dt.float32
    BF16 = mybir.dt.bfloat16
    U32 = mybir.dt.uint32
    I32 = mybir.dt.int32
    AF = mybir.ActivationFunctionType
    ALU = mybir.AluOpType
    AX = mybir.AxisListType

    nc = bacc.Bacc("TRN2", target_bir_lowering=False, debug=False, num_devices=NCORES)

    # ---- external inputs (per-core shards prepared on host) ----
    wt_bf = nc.dram_tensor("wt_bf", [128, NK, VSH], BF16, kind="ExternalInput")
    bias_bf = nc.dram_tensor("bias_bf", [1, VSH], BF16, kind="ExternalInput")
    w_aug = nc.dram_tensor("w_aug", [VSH, 1032], F32, kind="ExternalInput")
    wemb = nc.dram_tensor("wemb", [V, 1024], F32, kind="ExternalInput")
    we_lhsT = nc.dram_tensor("we_lhsT", [128, 3, NK, 128], F32, kind="ExternalInput")
    whh_lhsT = nc.dram_tensor("whh_lhsT", [128, 3, NK, 128], F32, kind="ExternalInput")
    cT_in = nc.dram_tensor("cT_in", [128, 3, 64], F32, kind="ExternalInput")
    bhh_n_in = nc.dram_tensor("bhh_n_in", [128, 1], F32, kind="ExternalInput")
    h0_own_in = nc.dram_tensor("h0_own_in", [128, 64], F32, kind="ExternalInput")
    hT0_in = nc.dram_tensor("hT0_in", [128, NK, 64], F32, kind="ExternalInput")
    haug0_in = nc.dram_tensor("haug0_in", [64, 1032], F32, kind="ExternalInput")
    coff_in = nc.dram_tensor("coff_in", [64, 1], F32, kind="ExternalInput")

    out = nc.dram_tensor("out", [64, steps], I32, kind="ExternalOutput")

    # ---- collective DRAM buffers (double buffered) ----
    ag1_in = [nc.dram_tensor(f"ag1_in{i}", [64, 2], F32) for i in range(2)]
    ag1_out = [nc.dram_tensor(f"ag1_out{i}", [8, 64, 2], F32, addr_space="Shared") for i in range(2)]
    ag2_in = [nc.dram_tensor(f"ag2_in{i}", [128, 64], F32) for i in range(2)]
    ag2_out = [nc.dram_tensor(f"ag2_out{i}", [8, 128, 64], F32, addr_space="Shared") for i in range(2)]

    from contextlib import ExitStack
    ctx = ExitStack()
    with ctx:
        tc = ctx.enter_context(tile.TileContext(nc))

        # ---- sbuf tensors ----
        wt_sb = nc.alloc_sbuf_tensor("wt_sb", [128, NK, VSH], BF16)
        bias_sb = nc.alloc_sbuf_tensor("bias_sb", [1, VSH], BF16)
        ones_sb = nc.alloc_sbuf_tensor("ones_sb", [1, 64], BF16)
        we_sb = nc.alloc_sbuf_tensor("we_sb", [128, 3, NK, 128], F32)
        whh_sb = nc.alloc_sbuf_tensor("whh_sb", [128, 3, NK, 128], F32)
        cT_sb = nc.alloc_sbuf_tensor("cT_sb", [128, 3, 64], F32)
        bhhn_sb = nc.alloc_sbuf_tensor("bhhn_sb", [128, 1], F32)
        coff_sb = nc.alloc_sbuf_tensor("coff_sb", [64, 1], F32)
        ident64 = nc.alloc_sbuf_tensor("ident64", [64, 64], F32)
        ident128 = nc.alloc_sbuf_tensor("ident128", [128, 128], F32)

        hT = nc.alloc_sbuf_tensor("hT", [128, NK, 64], F32)
        hT_bf = nc.alloc_sbuf_tensor("hT_bf", [128, NK, 64], BF16)
        h_aug = nc.alloc_sbuf_tensor("h_aug", [64, 1032], F32)
        h_own = nc.alloc_sbuf_tensor("h_own", [128, 64], F32)
        hnew = nc.alloc_sbuf_tensor("hnew", [128, 64], F32)
        embT = nc.alloc_sbuf_tensor("embT", [128, NK, 64], F32)
        emb_sb = nc.alloc_sbuf_tensor("emb_sb", [64, 1024], F32)

        logits = nc.alloc_sbuf_tensor("logits", [64, VSH], F32)
        maxv = nc.alloc_sbuf_tensor("maxv", [64, 8], F32)
        maxi = nc.alloc_sbuf_tensor("maxi", [64, 8], U32)
        maxi_f = nc.alloc_sbuf_tensor("maxi_f", [64, KCAND], F32)
        g4 = nc.alloc_sbuf_tensor("g4", [64, KCAND, 1032], F32)
        resc = nc.alloc_sbuf_tensor("resc", [64, KCAND], F32)

        rmax = nc.alloc_sbuf_tensor("rmax", [64, 1], F32)
        rtmp = nc.alloc_sbuf_tensor("rtmp", [64, KCAND], F32)
        rmask = nc.alloc_sbuf_tensor("rmask", [64, KCAND], F32)
        lidx = nc.alloc_sbuf_tensor("lidx", [64, 1], F32)
        agin_sb = nc.alloc_sbuf_tensor("agin_sb", [64, 2], F32)
        gg = nc.alloc_sbuf_tensor("gg", [64, 8, 2], F32)
        gmax = nc.alloc_sbuf_tensor("gmax", [64, 1], F32)
        gmask = nc.alloc_sbuf_tensor("gmask", [64, 8], F32)
        gtmp = nc.alloc_sbuf_tensor("gtmp", [64, 8], F32)
        tokf = nc.alloc_sbuf_tensor("tokf", [64, 1], F32)
        toku = nc.alloc_sbuf_tensor("toku", [64, 1], U32)
        toks = nc.alloc_sbuf_tensor("toks", [64, steps], I32)

        r_sb = nc.alloc_sbuf_tensor("r_sb", [128, 64], F32)
        z_sb = nc.alloc_sbuf_tensor("z_sb", [128, 64], F32)
        n_sb = nc.alloc_sbuf_tensor("n_sb", [128, 64], F32)
        gt1 = nc.alloc_sbuf_tensor("gt1", [128, 64], F32)
        gt2 = nc.alloc_sbuf_tensor("gt2", [128, 64], F32)

        # ---- psum ----
        ps_scr = [ctx.enter_context(nc.psum_tensor(f"ps_scr{i}", [64, 512], F32)) for i in range(2)]
        ps_g = ctx.enter_context(nc.psum_tensor("ps_g", [128, 2, 64], F32))
        ps_ghn = ctx.enter_context(nc.psum_tensor("ps_ghn", [128, 64], F32))
        ps_gin = ctx.enter_context(nc.psum_tensor("ps_gin", [128, 64], F32))
        ps_e = ctx.enter_context(nc.psum_tensor("ps_e", [128, 512], F32))
        ps_h0 = ctx.enter_context(nc.psum_tensor("ps_h0", [64, 512], F32))
        ps_h1 = ctx.enter_context(nc.psum_tensor("ps_h1", [64, 512], F32))

        # ---- preamble: load everything ----
        nc.sync.dma_start(wt_sb[:], wt_bf[:])
        nc.sync.dma_start(bias_sb[:], bias_bf[:])
        nc.sync.dma_start(we_sb[:], we_lhsT[:])
        nc.sync.dma_start(whh_sb[:], whh_lhsT[:])
        nc.sync.dma_start(cT_sb[:], cT_in[:])
        nc.sync.dma_start(bhhn_sb[:], bhh_n_in[:])
        nc.sync.dma_start(coff_sb[:], coff_in[:])
        nc.sync.dma_start(h_own[:], h0_own_in[:])
        nc.sync.dma_start(hT[:], hT0_in[:])
        nc.sync.dma_start(h_aug[:], haug0_in[:])
        nc.vector.tensor_copy(hT_bf[:], hT[:])
        nc.vector.memset(ones_sb[:], 1.0)
        make_identity(nc, ident64[:])
        make_identity(nc, ident128[:])

        prev_gg_read = [None, None]   # for WAR dep two steps back (ag1)
        prev_hT_read = [None, None]   # (ag2)

        for t in range(steps):
            db = t % 2

            # ===== screen matmuls (bf16) + bias row =====
            if screen:
                voff = 0
                for vt, vsz in enumerate(VT_SIZES):
                    ps = ps_scr[vt % 2]
                    for k in range(NK):
                        nc.tensor.matmul(
                            ps[:, 0:vsz],
                            hT_bf[:, k, :],
                            wt_sb[:, k, voff:voff + vsz],
                            start=(k == 0), stop=False)
                    nc.tensor.matmul(
                        ps[:, 0:vsz],
                        ones_sb[:],
                        bias_sb[:, voff:voff + vsz],
                        start=False, stop=True)
                    nc.scalar.copy(logits[:, voff:voff + vsz], ps[:, 0:vsz])
                    voff += vsz

            # ===== GRU h-side matmuls (only need hT) — emitted early so the
            # TensorEngine stays busy during the argmax/AllGather window =====
            for g in range(2):
                for k in range(NK):
                    nc.tensor.matmul(
                        ps_g[:, g, :], whh_sb[:, g, k, :], hT[:, k, :],
                        start=(g == 0 and k == 0), stop=False)
            for k in range(NK):
                nc.tensor.matmul(
                    ps_ghn[:], whh_sb[:, 2, k, :], hT[:, k, :],
                    start=(k == 0), stop=(k == NK - 1))

            # ===== local top-8 =====
            if screen:
                nc.vector.max(out=maxv[:], in_=logits[:])
                nc.vector.max_index(out=maxi[:], in_max=maxv[:], in_values=logits[:])
            else:
                nc.vector.memset(maxv[:], 0.0)
                nc.vector.memset(maxi[:], 0)
            nc.vector.tensor_copy(maxi_f[:], maxi[:, 0:KCAND])

            # ===== gather candidate [W|b] rows + exact rescore =====
            if rescore:
                for j in range(KCAND):
                    nc.gpsimd.indirect_dma_start(
                        out=g4[:, j, :],
                        out_offset=None,
                        in_=w_aug[:],
                        in_offset=bass.IndirectOffsetOnAxis(ap=maxi[:, j:j + 1], axis=0),
                    )
                nc.vector.tensor_mul(
                    g4[:], g4[:],
                    h_aug[:].unsqueeze(1).to_broadcast([64, KCAND, 1032]))
                nc.vector.tensor_reduce(resc[:], g4[:], axis=AX.X, op=ALU.add)
            else:
                nc.vector.tensor_copy(resc[:], maxv[:, 0:KCAND])

            # ===== local argmax of rescored (lowest global idx on ties) =====
            nc.vector.tensor_reduce(rmax[:], resc[:], axis=AX.X, op=ALU.max)
            nc.vector.tensor_scalar(rmask[:], resc[:], rmax[:, 0:1], None, op0=ALU.is_equal)
            nc.vector.tensor_scalar_add(rtmp[:], maxi_f[:], coff_sb[:, 0:1])   # global idx
            nc.vector.tensor_scalar_add(rtmp[:], rtmp[:], -BIG)
            nc.vector.tensor_mul(rtmp[:], rtmp[:], rmask[:])
            nc.vector.tensor_scalar_add(rtmp[:], rtmp[:], BIG)
            nc.vector.tensor_reduce(lidx[:], rtmp[:], axis=AX.X, op=ALU.min)
            nc.vector.tensor_copy(agin_sb[:, 0:1], rmax[:])
            nc.vector.tensor_copy(agin_sb[:, 1:2], lidx[:])

            # ===== AllGather candidates =====
            w1 = nc.sync.dma_start(ag1_in[db][:], agin_sb[:])
            if collectives:
                cc1 = nc.gpsimd.collective_compute(
                    "AllGather", ALU.bypass,
                    replica_groups=[list(range(NCORES))],
                    ins=[ag1_in[db][:]], outs=[ag1_out[db][:]],
                )
            else:
                cc1 = nc.sync.dma_start(ag1_out[db][0], ag1_in[db][:])
            add_dep_helper(cc1.ins, w1.ins, True, "ag1 after input write")
            if prev_gg_read[db] is not None:
                add_dep_helper(cc1.ins, prev_gg_read[db].ins, True, "ag1 WAR")
            r1 = nc.sync.dma_start(
                gg[:],
                bass.AP(ag1_out[db], 0, [[2, 64], [128, 8], [1, 2]]),
            )
            add_dep_helper(r1.ins, cc1.ins, True, "gg read after ag1")
            prev_gg_read[db] = r1

            # ===== global argmax combine =====
            nc.vector.tensor_reduce(gmax[:], gg[:, :, 0], axis=AX.X, op=ALU.max)
            nc.vector.tensor_scalar(gmask[:], gg[:, :, 0], gmax[:, 0:1], None, op0=ALU.is_equal)
            nc.vector.tensor_scalar_add(gtmp[:], gg[:, :, 1], -BIG)
            nc.vector.tensor_mul(gtmp[:], gtmp[:], gmask[:])
            nc.vector.tensor_scalar_add(gtmp[:], gtmp[:], BIG)
            nc.vector.tensor_reduce(tokf[:], gtmp[:], axis=AX.X, op=ALU.min)
            # clamp to V-1 so the emb gather can't go OOB even with garbage
            # inputs (timing variants); identity for any valid token id
            nc.vector.tensor_scalar(tokf[:], tokf[:], float(V - 1), None, op0=ALU.min)
            nc.vector.tensor_copy(toku[:], tokf[:])
            nc.vector.tensor_copy(toks[:, t:t + 1], tokf[:])

            # ===== embedding gather + transpose =====
            if embgather:
                nc.gpsimd.indirect_dma_start(
                    out=emb_sb[:],
                    out_offset=None,
                    in_=wemb[:],
                    in_offset=bass.IndirectOffsetOnAxis(ap=toku[:, 0:1], axis=0),
                )
            else:
                nc.sync.dma_start(emb_sb[:], wemb[0:64, :])
            for k in range(NK):
                nc.tensor.transpose(ps_e[:, k * 64:(k + 1) * 64],
                                    emb_sb[:, k * 128:(k + 1) * 128], ident64[:])
                nc.scalar.copy(embT[:, k, :], ps_e[:, k * 64:(k + 1) * 64])

            # ===== GRU emb-side matmuls (gh side was issued just after the
            # screen; these join the same psum accumulation groups) =====
            for g in range(2):
                for k in range(NK):
                    nc.tensor.matmul(
                        ps_g[:, g, :], we_sb[:, g, k, :], embT[:, k, :],
                        start=False, stop=(g == 1 and k == NK - 1))
            for k in range(NK):
                nc.tensor.matmul(
                    ps_gin[:], we_sb[:, 2, k, :], embT[:, k, :],
                    start=(k == 0), stop=(k == NK - 1))

            # ===== gates =====
            # r = sigmoid(gi_r + gh_r + c_r)  via exp/recip
            nc.vector.tensor_add(gt1[:], ps_g[:, 0, :], cT_sb[:, 0, :])
            nc.scalar.activation(gt2[:], gt1[:], AF.Exp, scale=-1.0)
            nc.vector.tensor_scalar_add(gt2[:], gt2[:], 1.0)
            nc.vector.reciprocal(r_sb[:], gt2[:])
            # z
            nc.vector.tensor_add(gt1[:], ps_g[:, 1, :], cT_sb[:, 1, :])
            nc.scalar.activation(gt2[:], gt1[:], AF.Exp, scale=-1.0)
            nc.vector.tensor_scalar_add(gt2[:], gt2[:], 1.0)
            nc.vector.reciprocal(z_sb[:], gt2[:])
            # n = tanh(gi_n + c_n + r * (gh_n + bhh_n))
            nc.vector.tensor_scalar_add(gt1[:], ps_ghn[:], bhhn_sb[:, 0:1])
            nc.vector.tensor_mul(gt1[:], gt1[:], r_sb[:])
            nc.vector.tensor_add(gt1[:], gt1[:], ps_gin[:])
            nc.vector.tensor_add(gt1[:], gt1[:], cT_sb[:, 2, :])
            nc.scalar.activation(n_sb[:], gt1[:], AF.Tanh)
            # h_new = n + z * (h_own - n)
            nc.vector.tensor_sub(gt1[:], h_own[:], n_sb[:])
            nc.vector.tensor_mul(gt1[:], gt1[:], z_sb[:])
            nc.vector.tensor_add(hnew[:], gt1[:], n_sb[:])
            nc.vector.tensor_copy(h_own[:], hnew[:])

            # ===== AllGather h chunks =====
            w2 = nc.sync.dma_start(ag2_in[db][:], hnew[:])
            if collectives:
                cc2 = nc.gpsimd.collective_compute(
                    "AllGather", ALU.bypass,
                    replica_groups=[list(range(NCORES))],
                    ins=[ag2_in[db][:]], outs=[ag2_out[db][:]],
                )
            else:
                cc2 = nc.sync.dma_start(ag2_out[db][0], ag2_in[db][:])
            add_dep_helper(cc2.ins, w2.ins, True, "ag2 after input write")
            if prev_hT_read[db] is not None:
                add_dep_helper(cc2.ins, prev_hT_read[db].ins, True, "ag2 WAR")
            if t < steps - 1:
                r2 = nc.sync.dma_start(
                    hT[:],
                    bass.AP(ag2_out[db], 0, [[64, 128], [8192, 8], [1, 64]]),
                )
                add_dep_helper(r2.ins, cc2.ins, True, "hT read after ag2")
                prev_hT_read[db] = r2
                nc.vector.tensor_copy(hT_bf[:], hT[:])
                # rebuild h_aug (batch-major h) via PE transposes
                for k in range(NK):
                    ps_h = ps_h0 if k < 4 else ps_h1
                    kk = k % 4
                    nc.tensor.transpose(ps_h[:, kk * 128:(kk + 1) * 128],
                                        hT[:, k, :], ident128[:])
                    nc.scalar.copy(h_aug[:, k * 128:(k + 1) * 128],
                                   ps_h[:, kk * 128:(kk + 1) * 128])

        nc.sync.dma_start(out[:], toks[:])

    nc.compile()
    return nc


def prep_inputs(M, questions, word_embedding, W_out, b_out, W_ih, W_hh, b_ih, b_hh):
    """Host-side shard prep. All args np.float32 arrays."""
    f32 = np.float32
    M = np.asarray(M, f32); questions = np.asarray(questions, f32)
    word_embedding = np.ascontiguousarray(np.asarray(word_embedding, f32))
    W_out = np.asarray(W_out, f32); b_out = np.asarray(b_out, f32)
    W_ih = np.asarray(W_ih, f32); W_hh = np.asarray(W_hh, f32)
    b_ih = np.asarray(b_ih, f32); b_hh = np.asarray(b_hh, f32)

    W_pad = np.zeros((VPAD, H), f32)
    W_pad[:V] = W_out
    b_pad = np.full((VPAD,), PAD_BIAS, f32)
    b_pad[:V] = b_out

    h0 = M[:, 0, :]                      # [64, 1024]
    q = questions[:, 0, :]               # [64, 1024]
    qW = (q.astype(np.float64) @ W_ih[:, 1024:].astype(np.float64).T).astype(f32)  # [64, 3072]

    hT0 = np.ascontiguousarray(h0.T)     # [1024, 64]
    hT0_in = hT0.reshape(NK, 128, 64).transpose(1, 0, 2)  # [128, NK, 64]
    haug0 = np.zeros((64, 1032), f32)
    haug0[:, :1024] = h0
    haug0[:, 1024] = 1.0

    in_maps = []
    for c in range(NCORES):
        rows = slice(c * VSH, (c + 1) * VSH)
        Wc = W_pad[rows]                                  # [6400, 1024]
        # wt_bf [128, NK, VSH]: [p, k, v] = Wc[v, k*128+p]
        wt = Wc.T.reshape(NK, 128, VSH)                   # [k, p, v] = Wc[v, k*128+p]
        wt_bf = np.ascontiguousarray(wt.transpose(1, 0, 2)).astype(ml_dtypes.bfloat16)
        bias_bf = b_pad[rows].reshape(1, VSH).astype(ml_dtypes.bfloat16)
        w_aug = np.zeros((VSH, 1032), f32)
        w_aug[:, :1024] = Wc
        w_aug[:, 1024] = b_pad[rows]

        gr = slice(c * 128, (c + 1) * 128)
        # We rows for gates r/z/n: W_ih[g*1024 + gr, :1024]
        we = np.stack([W_ih[g * 1024 + c * 128: g * 1024 + (c + 1) * 128, :1024] for g in range(3)])   # [3, 128m, 1024]
        # we_lhsT [128p, 3, NK, 128m] = we[g, m, k*128+p]
        we_lhsT = np.ascontiguousarray(we.reshape(3, 128, NK, 128).transpose(3, 0, 2, 1))
        whh = np.stack([W_hh[g * 1024 + c * 128: g * 1024 + (c + 1) * 128, :] for g in range(3)])
        whh_lhsT = np.ascontiguousarray(whh.reshape(3, 128, NK, 128).transpose(3, 0, 2, 1))

        # cT [128p, 3, 64b]
        cT = np.zeros((128, 3, 64), f32)
        for g in range(3):
            const = qW[:, g * 1024 + c * 128: g * 1024 + (c + 1) * 128] + b_ih[g * 1024 + gr.start: g * 1024 + gr.stop]
            if g < 2:
                const = const + b_hh[g * 1024 + gr.start: g * 1024 + gr.stop]
            cT[:, g, :] = const.T
        bhh_n = b_hh[2048 + gr.start: 2048 + gr.stop].reshape(128, 1)

        h0_own = np.ascontiguousarray(h0[:, gr].T)        # [128, 64]
        coff = np.full((64, 1), c * VSH, f32)

        in_maps.append({
            "wt_bf": wt_bf,
            "bias_bf": bias_bf,
            "w_aug": w_aug,
            "wemb": word_embedding,
            "we_lhsT": we_lhsT,
            "whh_lhsT": whh_lhsT,
            "cT_in": cT,
            "bhh_n_in": bhh_n,
            "h0_own_in": h0_own,
            "hT0_in": np.ascontiguousarray(hT0_in),
            "haug0_in": haug0,
            "coff_in": coff,
        })
    return in_maps


def build_v2(steps=T, max_from_sbuf=False, idx_on_vector=False):
    """Optimized step body:
    - per-vtile top-8 (max/max_index) read PSUM directly, hidden under the
      screen matmuls; no [64,6400] logits buffer or its copies
    - candidate index extraction via is_equal + fused tensor_tensor_reduce
      min-tricks (values carry idx-BIG so min() breaks ties to lowest idx)
    - rescore dots fused (mult+add-reduce in one DVE op per candidate)
    - sigmoid gates via the activation table (validated vs f32 reference)
    """
    import concourse.bass as bass
    import concourse.bacc as bacc
    import concourse.mybir as mybir
    from concourse import tile
    from concourse.tile_rust import add_dep_helper
    from concourse.masks import make_identity

    F32 = mybir.dt.float32
    BF16 = mybir.dt.bfloat16
    U32 = mybir.dt.uint32
    I32 = mybir.dt.int32
    AF = mybir.ActivationFunctionType
    ALU = mybir.AluOpType
    AX = mybir.AxisListType

    NT = len(VT_SIZES)          # 13 vtiles
    NC8 = NT * 8                # 104 candidate slots

    nc = bacc.Bacc("TRN2", target_bir_lowering=False, debug=False, num_devices=NCORES)

    wt_bf = nc.dram_tensor("wt_bf", [128, NK, VSH], BF16, kind="ExternalInput")
    bias_bf = nc.dram_tensor("bias_bf", [1, VSH], BF16, kind="ExternalInput")
    w_aug = nc.dram_tensor("w_aug", [VSH, 1032], F32, kind="ExternalInput")
    wemb = nc.dram_tensor("wemb", [V, 1024], F32, kind="ExternalInput")
    we_lhsT = nc.dram_tensor("we_lhsT", [128, 3, NK, 128], F32, kind="ExternalInput")
    whh_lhsT = nc.dram_tensor("whh_lhsT", [128, 3, NK, 128], F32, kind="ExternalInput")
    cT_in = nc.dram_tensor("cT_in", [128, 3, 64], F32, kind="ExternalInput")
    bhh_n_in = nc.dram_tensor("bhh_n_in", [128, 1], F32, kind="ExternalInput")
    h0_own_in = nc.dram_tensor("h0_own_in", [128, 64], F32, kind="ExternalInput")
    hT0_in = nc.dram_tensor("hT0_in", [128, NK, 64], F32, kind="ExternalInput")
    haug0_in = nc.dram_tensor("haug0_in", [64, 1032], F32, kind="ExternalInput")
    coff_in = nc.dram_tensor("coff_in", [64, 1], F32, kind="ExternalInput")

    out = nc.dram_tensor("out", [64, steps], I32, kind="ExternalOutput")

    ag1_in = [nc.dram_tensor(f"ag1_in{i}", [64, 2], F32) for i in range(2)]
    ag1_out = [nc.dram_tensor(f"ag1_out{i}", [8, 64, 2], F32, addr_space="Shared") for i in range(2)]
    ag2_in = [nc.dram_tensor(f"ag2_in{i}", [128, 64], F32) for i in range(2)]
    ag2_out = [nc.dram_tensor(f"ag2_out{i}", [8, 128, 64], F32, addr_space="Shared") for i in range(2)]

    from contextlib import ExitStack
    ctx = ExitStack()
    with ctx:
        tc = ctx.enter_context(tile.TileContext(nc))

        wt_sb = nc.alloc_sbuf_tensor("wt_sb", [128, NK, VSH], BF16)
        bias_sb = nc.alloc_sbuf_tensor("bias_sb", [1, VSH], BF16)
        ones_sb = nc.alloc_sbuf_tensor("ones_sb", [1, 64], BF16)
        we_sb = nc.alloc_sbuf_tensor("we_sb", [128, 3, NK, 128], F32)
        whh_sb = nc.alloc_sbuf_tensor("whh_sb", [128, 3, NK, 128], F32)
        cT_sb = nc.alloc_sbuf_tensor("cT_sb", [128, 3, 64], F32)
        bhhn_sb = nc.alloc_sbuf_tensor("bhhn_sb", [128, 1], F32)
        coff_sb = nc.alloc_sbuf_tensor("coff_sb", [64, 1], F32)
        ident64 = nc.alloc_sbuf_tensor("ident64", [64, 64], F32)
        ident128 = nc.alloc_sbuf_tensor("ident128", [128, 128], F32)

        hT = nc.alloc_sbuf_tensor("hT", [128, NK, 64], F32)
        hT_bf = nc.alloc_sbuf_tensor("hT_bf", [128, NK, 64], BF16)
        h_aug = nc.alloc_sbuf_tensor("h_aug", [64, 1032], F32)
        h_own = nc.alloc_sbuf_tensor("h_own", [128, 64], F32)
        hnew = nc.alloc_sbuf_tensor("hnew", [128, 64], F32)
        embT = nc.alloc_sbuf_tensor("embT", [128, NK, 64], F32)
        emb_sb = nc.alloc_sbuf_tensor("emb_sb", [64, 1024], F32)

        maxv_all = nc.alloc_sbuf_tensor("maxv_all", [64, NC8], F32)
        maxi8 = nc.alloc_sbuf_tensor("maxi8", [64, NC8], U32)
        vidxB = nc.alloc_sbuf_tensor("vidxB", [64, NC8], F32)
        voffB = nc.alloc_sbuf_tensor("voffB", [64, NT], F32)
        lg_sb = [nc.alloc_sbuf_tensor(f"lg_sb{i}", [64, 512], F32) for i in range(2)] \
            if max_from_sbuf else None
        gmax8 = nc.alloc_sbuf_tensor("gmax8", [64, 8], F32)
        maskc = nc.alloc_sbuf_tensor("maskc", [64, NC8], F32)
        ttr_scr = nc.alloc_sbuf_tensor("ttr_scr", [64, NC8], F32)
        candB = nc.alloc_sbuf_tensor("candB", [64, KCAND], F32)
        cand_f = nc.alloc_sbuf_tensor("cand_f", [64, KCAND], F32)
        cand_u = nc.alloc_sbuf_tensor("cand_u", [64, KCAND], U32)
        candGB = nc.alloc_sbuf_tensor("candGB", [64, KCAND], F32)
        g4 = nc.alloc_sbuf_tensor("g4", [64, KCAND, 1032], F32)
        g4s = nc.alloc_sbuf_tensor("g4s", [64, 1032], F32)
        resc = nc.alloc_sbuf_tensor("resc", [64, KCAND], F32)

        rmax = nc.alloc_sbuf_tensor("rmax", [64, 1], F32)
        rmask = nc.alloc_sbuf_tensor("rmask", [64, KCAND], F32)
        rtt_scr = nc.alloc_sbuf_tensor("rtt_scr", [64, KCAND], F32)
        lidxB = nc.alloc_sbuf_tensor("lidxB", [64, 1], F32)
        agin_sb = nc.alloc_sbuf_tensor("agin_sb", [64, 2], F32)
        gg = nc.alloc_sbuf_tensor("gg", [64, 8, 2], F32)
        gmax = nc.alloc_sbuf_tensor("gmax", [64, 1], F32)
        gmask = nc.alloc_sbuf_tensor("gmask", [64, 8], F32)
        gtt_scr = nc.alloc_sbuf_tensor("gtt_scr", [64, 8], F32)
        tokB = nc.alloc_sbuf_tensor("tokB", [64, 1], F32)
        tokf = nc.alloc_sbuf_tensor("tokf", [64, 1], F32)
        toku = nc.alloc_sbuf_tensor("toku", [64, 1], U32)
        toks = nc.alloc_sbuf_tensor("toks", [64, steps], I32)

        r_sb = nc.alloc_sbuf_tensor("r_sb", [128, 64], F32)
        z_sb = nc.alloc_sbuf_tensor("z_sb", [128, 64], F32)
        n_sb = nc.alloc_sbuf_tensor("n_sb", [128, 64], F32)
        gt1 = nc.alloc_sbuf_tensor("gt1", [128, 64], F32)
        gt2 = nc.alloc_sbuf_tensor("gt2", [128, 64], F32)
        gt3 = nc.alloc_sbuf_tensor("gt3", [128, 64], F32)

        ps_scr = [ctx.enter_context(nc.psum_tensor(f"ps_scr{i}", [64, 512], F32)) for i in range(2)]
        ps_g = ctx.enter_context(nc.psum_tensor("ps_g", [128, 2, 64], F32))
        ps_ghn = ctx.enter_context(nc.psum_tensor("ps_ghn", [128, 64], F32))
        ps_gin = ctx.enter_context(nc.psum_tensor("ps_gin", [128, 64], F32))
        ps_e = ctx.enter_context(nc.psum_tensor("ps_e", [128, 512], F32))
        ps_h0 = ctx.enter_context(nc.psum_tensor("ps_h0", [64, 512], F32))
        ps_h1 = ctx.enter_context(nc.psum_tensor("ps_h1", [64, 512], F32))

        # ---- preamble ----
        nc.sync.dma_start(wt_sb[:], wt_bf[:])
        nc.sync.dma_start(bias_sb[:], bias_bf[:])
        nc.sync.dma_start(we_sb[:], we_lhsT[:])
        nc.sync.dma_start(whh_sb[:], whh_lhsT[:])
        nc.sync.dma_start(cT_sb[:], cT_in[:])
        nc.sync.dma_start(bhhn_sb[:], bhh_n_in[:])
        nc.sync.dma_start(coff_sb[:], coff_in[:])
        nc.sync.dma_start(h_own[:], h0_own_in[:])
        nc.sync.dma_start(hT[:], hT0_in[:])
        nc.sync.dma_start(h_aug[:], haug0_in[:])
        nc.vector.tensor_copy(hT_bf[:], hT[:])
        nc.vector.memset(ones_sb[:], 1.0)
        voff0 = 0
        for vt, vsz in enumerate(VT_SIZES):
            nc.vector.memset(voffB[:, vt:vt + 1], float(voff0 - BIG))
            voff0 += vsz
        make_identity(nc, ident64[:])
        make_identity(nc, ident128[:])

        prev_gg_read = [None, None]
        prev_hT_read = [None, None]

        for t in range(steps):
            db = t % 2

            # ===== screen + hidden per-tile top-8 =====
            voff = 0
            for vt, vsz in enumerate(VT_SIZES):
                ps = ps_scr[vt % 2]
                for k in range(NK):
                    nc.tensor.matmul(
                        ps[:, 0:vsz],
                        hT_bf[:, k, :],
                        wt_sb[:, k, voff:voff + vsz],
                        start=(k == 0), stop=False)
                nc.tensor.matmul(
                    ps[:, 0:vsz],
                    ones_sb[:],
                    bias_sb[:, voff:voff + vsz],
                    start=False, stop=True)
                sl = slice(vt * 8, vt * 8 + 8)
                if max_from_sbuf:
                    lg = lg_sb[vt % 2]
                    nc.scalar.copy(lg[:, 0:vsz], ps[:, 0:vsz])
                    src = lg[:, 0:vsz]
                else:
                    src = ps[:, 0:vsz]
                nc.vector.max(out=maxv_all[:, sl], in_=src)
                nc.vector.max_index(out=maxi8[:, sl], in_max=maxv_all[:, sl],
                                    in_values=src)
                if idx_on_vector:
                    nc.vector.tensor_copy(vidxB[:, sl], maxi8[:, sl])
                    nc.vector.tensor_scalar_add(vidxB[:, sl], vidxB[:, sl],
                                                voffB[:, vt:vt + 1])
                else:
                    # u32 idx -> f32 with +voff-BIG, on the (idle) scalar engine
                    nc.scalar.activation(vidxB[:, sl], maxi8[:, sl], AF.Identity,
                                         bias=voffB[:, vt:vt + 1], scale=1.0)
                voff += vsz

            # ===== GRU h-side matmuls keep the PE busy during argmax =====
            for g in range(2):
                for k in range(NK):
                    nc.tensor.matmul(
                        ps_g[:, g, :], whh_sb[:, g, k, :], hT[:, k, :],
                        start=(g == 0 and k == 0), stop=False)
            for k in range(NK):
                nc.tensor.matmul(
                    ps_ghn[:], whh_sb[:, 2, k, :], hT[:, k, :],
                    start=(k == 0), stop=(k == NK - 1))

            # ===== global top-4 of the 104 per-tile candidates =====
            # (values carry idx-BIG via vidxB so min() breaks ties to the
            # lowest index; non-matching slots contribute 0 > any match)
            nc.vector.max(out=gmax8[:], in_=maxv_all[:])
            for j in range(KCAND):
                nc.vector.tensor_scalar(maskc[:], maxv_all[:], gmax8[:, j:j + 1],
                                        None, op0=ALU.is_equal)
                nc.vector.tensor_mul(ttr_scr[:], maskc[:], vidxB[:])
                nc.vector.tensor_reduce(candB[:, j:j + 1], ttr_scr[:],
                                        axis=AX.X, op=ALU.min)
                nc.vector.tensor_scalar_add(cand_f[:, j:j + 1], candB[:, j:j + 1], BIG)
                nc.vector.tensor_copy(cand_u[:, j:j + 1], cand_f[:, j:j + 1])
                nc.gpsimd.indirect_dma_start(
                    out=g4[:, j, :],
                    out_offset=None,
                    in_=w_aug[:],
                    in_offset=bass.IndirectOffsetOnAxis(ap=cand_u[:, j:j + 1], axis=0),
                )

            # ===== exact rescore: mul on VectorE, sum on ScalarE =====
            for j in range(KCAND):
                nc.vector.tensor_mul(g4[:, j, :], g4[:, j, :], h_aug[:])
                nc.scalar.activation(g4s[:], g4[:, j, :], AF.Identity,
                                     accum_out=resc[:, j:j + 1])

            # ===== local argmax (lowest global idx on ties) =====
            nc.vector.tensor_scalar_add(candGB[:], candB[:], coff_sb[:, 0:1])
            nc.vector.tensor_reduce(rmax[:], resc[:], axis=AX.X, op=ALU.max)
            nc.vector.tensor_scalar(rmask[:], resc[:], rmax[:, 0:1], None, op0=ALU.is_equal)
            nc.vector.tensor_mul(rtt_scr[:], rmask[:], candGB[:])
            nc.vector.tensor_reduce(lidxB[:], rtt_scr[:], axis=AX.X, op=ALU.min)
            nc.vector.tensor_copy(agin_sb[:, 0:1], rmax[:])
            nc.vector.tensor_copy(agin_sb[:, 1:2], lidxB[:])

            # ===== AllGather candidates =====
            w1 = nc.sync.dma_start(ag1_in[db][:], agin_sb[:])
            cc1 = nc.gpsimd.collective_compute(
                "AllGather", ALU.bypass,
                replica_groups=[list(range(NCORES))],
                ins=[ag1_in[db][:]], outs=[ag1_out[db][:]],
            )
            add_dep_helper(cc1.ins, w1.ins, True, "ag1 after input write")
            if prev_gg_read[db] is not None:
                add_dep_helper(cc1.ins, prev_gg_read[db].ins, True, "ag1 WAR")
            r1 = nc.sync.dma_start(
                gg[:],
                bass.AP(ag1_out[db], 0, [[2, 64], [128, 8], [1, 2]]),
            )
            add_dep_helper(r1.ins, cc1.ins, True, "gg read after ag1")
            prev_gg_read[db] = r1

            # ===== global argmax combine =====
            nc.vector.tensor_reduce(gmax[:], gg[:, :, 0], axis=AX.X, op=ALU.max)
            nc.vector.tensor_scalar(gmask[:], gg[:, :, 0], gmax[:, 0:1], None, op0=ALU.is_equal)
            nc.vector.tensor_mul(gtt_scr[:], gmask[:], gg[:, :, 1])
            nc.vector.tensor_reduce(tokB[:], gtt_scr[:], axis=AX.X, op=ALU.min)
            nc.vector.tensor_scalar_add(tokf[:], tokB[:], BIG)
            nc.vector.tensor_scalar(tokf[:], tokf[:], float(V - 1), None, op0=ALU.min)
            nc.vector.tensor_copy(toku[:], tokf[:])
            nc.vector.tensor_copy(toks[:, t:t + 1], tokf[:])

            # ===== embedding gather + transpose =====
            nc.gpsimd.indirect_dma_start(
                out=emb_sb[:],
                out_offset=None,
                in_=wemb[:],
                in_offset=bass.IndirectOffsetOnAxis(ap=toku[:, 0:1], axis=0),
            )
            for k in range(NK):
                nc.tensor.transpose(ps_e[:, k * 64:(k + 1) * 64],
                                    emb_sb[:, k * 128:(k + 1) * 128], ident64[:])
                nc.scalar.copy(embT[:, k, :], ps_e[:, k * 64:(k + 1) * 64])

            # ===== GRU emb-side matmuls =====
            for g in range(2):
                for k in range(NK):
                    nc.tensor.matmul(
                        ps_g[:, g, :], we_sb[:, g, k, :], embT[:, k, :],
                        start=False, stop=(g == 1 and k == NK - 1))
            for k in range(NK):
                nc.tensor.matmul(
                    ps_gin[:], we_sb[:, 2, k, :], embT[:, k, :],
                    start=(k == 0), stop=(k == NK - 1))

            # ===== gates (sigmoid/tanh via activation table) =====
            nc.vector.tensor_add(gt1[:], ps_g[:, 0, :], cT_sb[:, 0, :])
            nc.scalar.activation(r_sb[:], gt1[:], AF.Sigmoid)
            nc.vector.tensor_add(gt2[:], ps_g[:, 1, :], cT_sb[:, 1, :])
            nc.scalar.activation(z_sb[:], gt2[:], AF.Sigmoid)
            nc.vector.tensor_scalar_add(gt3[:], ps_ghn[:], bhhn_sb[:, 0:1])
            nc.vector.tensor_mul(gt3[:], gt3[:], r_sb[:])
            nc.vector.tensor_add(gt3[:], gt3[:], ps_gin[:])
            nc.vector.tensor_add(gt3[:], gt3[:], cT_sb[:, 2, :])
            nc.scalar.activation(n_sb[:], gt3[:], AF.Tanh)
            nc.vector.tensor_sub(gt3[:], h_own[:], n_sb[:])
            nc.vector.tensor_mul(gt3[:], gt3[:], z_sb[:])
            nc.vector.tensor_add(hnew[:], gt3[:], n_sb[:])
            nc.vector.tensor_copy(h_own[:], hnew[:])

            # ===== AllGather h chunks =====
            w2 = nc.sync.dma_start(ag2_in[db][:], hnew[:])
            cc2 = nc.gpsimd.collective_compute(
                "AllGather", ALU.bypass,
                replica_groups=[list(range(NCORES))],
                ins=[ag2_in[db][:]], outs=[ag2_out[db][:]],
            )
            add_dep_helper(cc2.ins, w2.ins, True, "ag2 after input write")
            if prev_hT_read[db] is not None:
                add_dep_helper(cc2.ins, prev_hT_read[db].ins, True, "ag2 WAR")
            if t < steps - 1:
                r2 = nc.sync.dma_start(
                    hT[:],
                    bass.AP(ag2_out[db], 0, [[64, 128], [8192, 8], [1, 64]]),
                )
                add_dep_helper(r2.ins, cc2.ins, True, "hT read after ag2")
                prev_hT_read[db] = r2
                nc.vector.tensor_copy(hT_bf[:], hT[:])
                for k in range(NK):
                    ps_h = ps_h0 if k < 4 else ps_h1
                    kk = k % 4
                    nc.tensor.transpose(ps_h[:, kk * 128:(kk + 1) * 128],
                                        hT[:, k, :], ident128[:])
                    nc.scalar.copy(h_aug[:, k * 128:(k + 1) * 128],
                                   ps_h[:, kk * 128:(kk + 1) * 128])

        nc.sync.dma_start(out[:], toks[:])

    nc.compile()
    return nc


def build_v3(steps=T):
    """v2 plus:
    - k-outer screen in groups of 4 vtiles with per-chunk hT reads, so the
      next screen starts as soon as the first h chunk lands after ag2
    - ag1 payload transposed to [2,64] so collective reads are
      descriptor-light ([8,64] contiguous) and re-transposed on the PE
    - GRU psums consolidated into one bank to free banks for the 4-deep
      screen pipeline
    """
    import concourse.bass as bass
    import concourse.bacc as bacc
    import concourse.mybir as mybir
    from concourse import tile
    from concourse.tile_rust import add_dep_helper
    from concourse.masks import make_identity

    F32 = mybir.dt.float32
    BF16 = mybir.dt.bfloat16
    U32 = mybir.dt.uint32
    I32 = mybir.dt.int32
    AF = mybir.ActivationFunctionType
    ALU = mybir.AluOpType
    AX = mybir.AxisListType

    NT = len(VT_SIZES)
    NC8 = NT * 8
    GROUPS = [list(range(g, min(g + 4, NT))) for g in range(0, NT, 4)]
    VOFFS = []
    _vo = 0
    for vs in VT_SIZES:
        VOFFS.append(_vo)
        _vo += vs

    nc = bacc.Bacc("TRN2", target_bir_lowering=False, debug=False, num_devices=NCORES)

    wt_bf = nc.dram_tensor("wt_bf", [128, NK, VSH], BF16, kind="ExternalInput")
    bias_bf = nc.dram_tensor("bias_bf", [1, VSH], BF16, kind="ExternalInput")
    w_aug = nc.dram_tensor("w_aug", [VSH, 1032], F32, kind="ExternalInput")
    wemb = nc.dram_tensor("wemb", [V, 1024], F32, kind="ExternalInput")
    we_lhsT = nc.dram_tensor("we_lhsT", [128, 3, NK, 128], F32, kind="ExternalInput")
    whh_lhsT = nc.dram_tensor("whh_lhsT", [128, 3, NK, 128], F32, kind="ExternalInput")
    cT_in = nc.dram_tensor("cT_in", [128, 3, 64], F32, kind="ExternalInput")
    bhh_n_in = nc.dram_tensor("bhh_n_in", [128, 1], F32, kind="ExternalInput")
    h0_own_in = nc.dram_tensor("h0_own_in", [128, 64], F32, kind="ExternalInput")
    hT0_in = nc.dram_tensor("hT0_in", [128, NK, 64], F32, kind="ExternalInput")
    haug0_in = nc.dram_tensor("haug0_in", [64, 1032], F32, kind="ExternalInput")
    coff_in = nc.dram_tensor("coff_in", [64, 1], F32, kind="ExternalInput")

    out = nc.dram_tensor("out", [64, steps], I32, kind="ExternalOutput")

    ag1_in = [nc.dram_tensor(f"ag1_in{i}", [2, 64], F32) for i in range(2)]
    ag1_out = [nc.dram_tensor(f"ag1_out{i}", [8, 2, 64], F32, addr_space="Shared") for i in range(2)]
    ag2_in = [nc.dram_tensor(f"ag2_in{i}", [128, 64], F32) for i in range(2)]
    ag2_out = [nc.dram_tensor(f"ag2_out{i}", [8, 128, 64], F32, addr_space="Shared") for i in range(2)]

    from contextlib import ExitStack
    ctx = ExitStack()
    with ctx:
        tc = ctx.enter_context(tile.TileContext(nc))

        wt_sb = nc.alloc_sbuf_tensor("wt_sb", [128, NK, VSH], BF16)
        bias_sb = nc.alloc_sbuf_tensor("bias_sb", [1, VSH], BF16)
        ones_sb = nc.alloc_sbuf_tensor("ones_sb", [1, 64], BF16)
        we_sb = nc.alloc_sbuf_tensor("we_sb", [128, 3, NK, 128], F32)
        whh_sb = nc.alloc_sbuf_tensor("whh_sb", [128, 3, NK, 128], F32)
        cT_sb = nc.alloc_sbuf_tensor("cT_sb", [128, 3, 64], F32)
        bhhn_sb = nc.alloc_sbuf_tensor("bhhn_sb", [128, 1], F32)
        coff_sb = nc.alloc_sbuf_tensor("coff_sb", [64, 1], F32)
        ident64 = nc.alloc_sbuf_tensor("ident64", [64, 64], F32)
        ident128 = nc.alloc_sbuf_tensor("ident128", [128, 128], F32)

        hT = nc.alloc_sbuf_tensor("hT", [128, NK, 64], F32)
        hT_bf = nc.alloc_sbuf_tensor("hT_bf", [128, NK, 64], BF16)
        h_aug = nc.alloc_sbuf_tensor("h_aug", [64, 1032], F32)
        h_own = nc.alloc_sbuf_tensor("h_own", [128, 64], F32)
        hnew = nc.alloc_sbuf_tensor("hnew", [128, 64], F32)
        embT = nc.alloc_sbuf_tensor("embT", [128, NK, 64], F32)
        emb_sb = nc.alloc_sbuf_tensor("emb_sb", [64, 1024], F32)

        maxv_all = nc.alloc_sbuf_tensor("maxv_all", [64, NC8], F32)
        maxi8 = nc.alloc_sbuf_tensor("maxi8", [64, NC8], U32)
        vidxB = nc.alloc_sbuf_tensor("vidxB", [64, NC8], F32)
        voffB = nc.alloc_sbuf_tensor("voffB", [64, NT], F32)
        gmax8 = nc.alloc_sbuf_tensor("gmax8", [64, 8], F32)
        maskc = nc.alloc_sbuf_tensor("maskc", [64, NC8], F32)
        ttr_scr = nc.alloc_sbuf_tensor("ttr_scr", [64, NC8], F32)
        candB = nc.alloc_sbuf_tensor("candB", [64, KCAND], F32)
        cand_f = nc.alloc_sbuf_tensor("cand_f", [64, KCAND], F32)
        cand_u = nc.alloc_sbuf_tensor("cand_u", [64, KCAND], U32)
        candGB = nc.alloc_sbuf_tensor("candGB", [64, KCAND], F32)
        g4 = nc.alloc_sbuf_tensor("g4", [64, KCAND, 1032], F32)
        g4s = nc.alloc_sbuf_tensor("g4s", [64, 1032], F32)
        resc = nc.alloc_sbuf_tensor("resc", [64, KCAND], F32)

        rmax = nc.alloc_sbuf_tensor("rmax", [64, 1], F32)
        rmask = nc.alloc_sbuf_tensor("rmask", [64, KCAND], F32)
        rtt_scr = nc.alloc_sbuf_tensor("rtt_scr", [64, KCAND], F32)
        lidxB = nc.alloc_sbuf_tensor("lidxB", [64, 1], F32)
        agin_sb = nc.alloc_sbuf_tensor("agin_sb", [64, 2], F32)
        aginT_sb = nc.alloc_sbuf_tensor("aginT_sb", [2, 64], F32)
        gv8 = nc.alloc_sbuf_tensor("gv8", [8, 64], F32)
        gi8 = nc.alloc_sbuf_tensor("gi8", [8, 64], F32)
        gmaxv8 = nc.alloc_sbuf_tensor("gmaxv8", [64, 8], F32)
        gidx8 = nc.alloc_sbuf_tensor("gidx8", [64, 8], F32)
        gmax = nc.alloc_sbuf_tensor("gmax", [64, 1], F32)
        gmask = nc.alloc_sbuf_tensor("gmask", [64, 8], F32)
        gtt_scr = nc.alloc_sbuf_tensor("gtt_scr", [64, 8], F32)
        tokB = nc.alloc_sbuf_tensor("tokB", [64, 1], F32)
        tokf = nc.alloc_sbuf_tensor("tokf", [64, 1], F32)
        toku = nc.alloc_sbuf_tensor("toku", [64, 1], U32)
        toks = nc.alloc_sbuf_tensor("toks", [64, steps], I32)

        r_sb = nc.alloc_sbuf_tensor("r_sb", [128, 64], F32)
        z_sb = nc.alloc_sbuf_tensor("z_sb", [128, 64], F32)
        n_sb = nc.alloc_sbuf_tensor("n_sb", [128, 64], F32)
        gt1 = nc.alloc_sbuf_tensor("gt1", [128, 64], F32)
        gt2 = nc.alloc_sbuf_tensor("gt2", [128, 64], F32)
        gt3 = nc.alloc_sbuf_tensor("gt3", [128, 64], F32)

        ps_scr = [ctx.enter_context(nc.psum_tensor(f"ps_scr{i}", [64, 512], F32)) for i in range(4)]
        ps_gru = ctx.enter_context(nc.psum_tensor("ps_gru", [128, 4, 64], F32))
        ps_e = ctx.enter_context(nc.psum_tensor("ps_e", [128, 512], F32))
        ps_h0 = ctx.enter_context(nc.psum_tensor("ps_h0", [64, 512], F32))
        ps_h1 = ctx.enter_context(nc.psum_tensor("ps_h1", [64, 512], F32))

        # ---- preamble ----
        nc.sync.dma_start(wt_sb[:], wt_bf[:])
        nc.sync.dma_start(bias_sb[:], bias_bf[:])
        nc.sync.dma_start(we_sb[:], we_lhsT[:])
        nc.sync.dma_start(whh_sb[:], whh_lhsT[:])
        nc.sync.dma_start(cT_sb[:], cT_in[:])
        nc.sync.dma_start(bhhn_sb[:], bhh_n_in[:])
        nc.sync.dma_start(coff_sb[:], coff_in[:])
        nc.sync.dma_start(h_own[:], h0_own_in[:])
        nc.sync.dma_start(hT[:], hT0_in[:])
        nc.sync.dma_start(h_aug[:], haug0_in[:])
        nc.vector.tensor_copy(hT_bf[:], hT[:])
        nc.vector.memset(ones_sb[:], 1.0)
        for vt in range(NT):
            nc.vector.memset(voffB[:, vt:vt + 1], float(VOFFS[vt] - BIG))
        make_identity(nc, ident64[:])
        make_identity(nc, ident128[:])

        prev_gg_read = [None, None]   # list of read instrs per db
        prev_hT_read = [None, None]

        for t in range(steps):
            db = t % 2

            # ===== screen: groups of 4 vtiles, k-outer inside a group =====
            for grp in GROUPS:
                for k in range(NK):
                    for vt in grp:
                        nc.tensor.matmul(
                            ps_scr[vt % 4][:, 0:VT_SIZES[vt]],
                            hT_bf[:, k, :],
                            wt_sb[:, k, VOFFS[vt]:VOFFS[vt] + VT_SIZES[vt]],
                            start=(k == 0), stop=False)
                for vt in grp:
                    nc.tensor.matmul(
                        ps_scr[vt % 4][:, 0:VT_SIZES[vt]],
                        ones_sb[:],
                        bias_sb[:, VOFFS[vt]:VOFFS[vt] + VT_SIZES[vt]],
                        start=False, stop=True)
                for vt in grp:
                    vsz = VT_SIZES[vt]
                    sl = slice(vt * 8, vt * 8 + 8)
                    nc.vector.max(out=maxv_all[:, sl], in_=ps_scr[vt % 4][:, 0:vsz])
                    nc.vector.max_index(out=maxi8[:, sl], in_max=maxv_all[:, sl],
                                        in_values=ps_scr[vt % 4][:, 0:vsz])
                    nc.scalar.activation(vidxB[:, sl], maxi8[:, sl], AF.Identity,
                                         bias=voffB[:, vt:vt + 1], scale=1.0)

            # ===== GRU h-side matmuls keep the PE busy during argmax =====
            # ps_gru is ONE psum zero-region: a single accumulation group
            # spans all h-side and emb-side writes (start at the first
            # h-side matmul, stop at the last emb-side one).
            for g in range(2):
                for k in range(NK):
                    nc.tensor.matmul(
                        ps_gru[:, g, :], whh_sb[:, g, k, :], hT[:, k, :],
                        start=(g == 0 and k == 0), stop=False)
            for k in range(NK):
                nc.tensor.matmul(
                    ps_gru[:, 2, :], whh_sb[:, 2, k, :], hT[:, k, :],
                    start=False, stop=False)

            # ===== global top-4 of the 104 per-tile candidates =====
            nc.vector.max(out=gmax8[:], in_=maxv_all[:])
            for j in range(KCAND):
                nc.vector.tensor_scalar(maskc[:], maxv_all[:], gmax8[:, j:j + 1],
                                        None, op0=ALU.is_equal)
                nc.vector.tensor_mul(ttr_scr[:], maskc[:], vidxB[:])
                nc.vector.tensor_reduce(candB[:, j:j + 1], ttr_scr[:],
                                        axis=AX.X, op=ALU.min)
                nc.vector.tensor_scalar_add(cand_f[:, j:j + 1], candB[:, j:j + 1], BIG)
                nc.vector.tensor_copy(cand_u[:, j:j + 1], cand_f[:, j:j + 1])
                nc.gpsimd.indirect_dma_start(
                    out=g4[:, j, :],
                    out_offset=None,
                    in_=w_aug[:],
                    in_offset=bass.IndirectOffsetOnAxis(ap=cand_u[:, j:j + 1], axis=0),
                )

            # ===== exact rescore: mul on VectorE, sum on ScalarE =====
            for j in range(KCAND):
                nc.vector.tensor_mul(g4[:, j, :], g4[:, j, :], h_aug[:])
                nc.scalar.activation(g4s[:], g4[:, j, :], AF.Identity,
                                     accum_out=resc[:, j:j + 1])

            # ===== local argmax (lowest global idx on ties) =====
            nc.vector.tensor_scalar_add(candGB[:], candB[:], coff_sb[:, 0:1])
            nc.vector.tensor_reduce(rmax[:], resc[:], axis=AX.X, op=ALU.max)
            nc.vector.tensor_scalar(rmask[:], resc[:], rmax[:, 0:1], None, op0=ALU.is_equal)
            nc.vector.tensor_mul(rtt_scr[:], rmask[:], candGB[:])
            nc.vector.tensor_reduce(lidxB[:], rtt_scr[:], axis=AX.X, op=ALU.min)
            nc.vector.tensor_copy(agin_sb[:, 0:1], rmax[:])
            nc.vector.tensor_copy(agin_sb[:, 1:2], lidxB[:])
            # transpose [64,2] -> [2,64] so the collective transport is
            # contiguous on both sides
            nc.tensor.transpose(ps_e[0:2, 0:64], agin_sb[:], ident64[:])
            nc.scalar.copy(aginT_sb[:], ps_e[0:2, 0:64])

            # ===== AllGather candidates =====
            w1 = nc.sync.dma_start(ag1_in[db][:], aginT_sb[:])
            cc1 = nc.gpsimd.collective_compute(
                "AllGather", ALU.bypass,
                replica_groups=[list(range(NCORES))],
                ins=[ag1_in[db][:]], outs=[ag1_out[db][:]],
            )
            add_dep_helper(cc1.ins, w1.ins, True, "ag1 after input write")
            if prev_gg_read[db] is not None:
                for pr in prev_gg_read[db]:
                    add_dep_helper(cc1.ins, pr.ins, True, "ag1 WAR")
            # vals at rows 0,2,..14; idx at rows 1,3,..15 of [16,64]
            r1v = nc.sync.dma_start(
                gv8[:], bass.AP(ag1_out[db], 0, [[128, 8], [1, 64]]))
            r1i = nc.sync.dma_start(
                gi8[:], bass.AP(ag1_out[db], 64, [[128, 8], [1, 64]]))
            add_dep_helper(r1v.ins, cc1.ins, True, "gv8 read after ag1")
            add_dep_helper(r1i.ins, cc1.ins, True, "gi8 read after ag1")
            prev_gg_read[db] = [r1v, r1i]
            nc.tensor.transpose(ps_e[0:64, 64:72], gv8[:], ident64[0:8, 0:8])
            nc.tensor.transpose(ps_e[0:64, 72:80], gi8[:], ident64[0:8, 0:8])
            nc.scalar.copy(gmaxv8[:], ps_e[0:64, 64:72])
            nc.scalar.copy(gidx8[:], ps_e[0:64, 72:80])

            # ===== global argmax combine =====
            nc.vector.tensor_reduce(gmax[:], gmaxv8[:], axis=AX.X, op=ALU.max)
            nc.vector.tensor_scalar(gmask[:], gmaxv8[:], gmax[:, 0:1], None, op0=ALU.is_equal)
            nc.vector.tensor_mul(gtt_scr[:], gmask[:], gidx8[:])
            nc.vector.tensor_reduce(tokB[:], gtt_scr[:], axis=AX.X, op=ALU.min)
            nc.vector.tensor_scalar_add(tokf[:], tokB[:], BIG)
            nc.vector.tensor_scalar(tokf[:], tokf[:], float(V - 1), None, op0=ALU.min)
            nc.vector.tensor_copy(toku[:], tokf[:])
            nc.vector.tensor_copy(toks[:, t:t + 1], tokf[:])

            # ===== embedding gather + transpose =====
            nc.gpsimd.indirect_dma_start(
                out=emb_sb[:],
                out_offset=None,
                in_=wemb[:],
                in_offset=bass.IndirectOffsetOnAxis(ap=toku[:, 0:1], axis=0),
            )
            for k in range(NK):
                nc.tensor.transpose(ps_e[:, k * 64:(k + 1) * 64],
                                    emb_sb[:, k * 128:(k + 1) * 128], ident64[:])
                nc.scalar.copy(embT[:, k, :], ps_e[:, k * 64:(k + 1) * 64])

            # ===== GRU emb-side matmuls (same ps_gru group; single stop) =====
            for g in range(2):
                for k in range(NK):
                    nc.tensor.matmul(
                        ps_gru[:, g, :], we_sb[:, g, k, :], embT[:, k, :],
                        start=False, stop=False)
            for k in range(NK):
                nc.tensor.matmul(
                    ps_gru[:, 3, :], we_sb[:, 2, k, :], embT[:, k, :],
                    start=False, stop=(k == NK - 1))

            # ===== gates =====
            nc.vector.tensor_add(gt1[:], ps_gru[:, 0, :], cT_sb[:, 0, :])
            nc.scalar.activation(r_sb[:], gt1[:], AF.Sigmoid)
            nc.vector.tensor_add(gt2[:], ps_gru[:, 1, :], cT_sb[:, 1, :])
            nc.scalar.activation(z_sb[:], gt2[:], AF.Sigmoid)
            nc.vector.tensor_scalar_add(gt3[:], ps_gru[:, 2, :], bhhn_sb[:, 0:1])
            nc.vector.tensor_mul(gt3[:], gt3[:], r_sb[:])
            nc.vector.tensor_add(gt3[:], gt3[:], ps_gru[:, 3, :])
            nc.vector.tensor_add(gt3[:], gt3[:], cT_sb[:, 2, :])
            nc.scalar.activation(n_sb[:], gt3[:], AF.Tanh)
            nc.vector.tensor_sub(gt3[:], h_own[:], n_sb[:])
            nc.vector.tensor_mul(gt3[:], gt3[:], z_sb[:])
            nc.vector.tensor_add(hnew[:], gt3[:], n_sb[:])
            nc.vector.tensor_copy(h_own[:], hnew[:])

            # ===== AllGather h chunks; per-chunk reads feed the next screen =====
            w2 = nc.sync.dma_start(ag2_in[db][:], hnew[:])
            cc2 = nc.gpsimd.collective_compute(
                "AllGather", ALU.bypass,
                replica_groups=[list(range(NCORES))],
                ins=[ag2_in[db][:]], outs=[ag2_out[db][:]],
            )
            add_dep_helper(cc2.ins, w2.ins, True, "ag2 after input write")
            if prev_hT_read[db] is not None:
                for pr in prev_hT_read[db]:
                    add_dep_helper(cc2.ins, pr.ins, True, "ag2 WAR")
            if t < steps - 1:
                reads = []
                for k in range(NK):
                    rk = nc.sync.dma_start(hT[:, k, :], ag2_out[db][k])
                    add_dep_helper(rk.ins, cc2.ins, True, "hT chunk after ag2")
                    reads.append(rk)
                    nc.vector.tensor_copy(hT_bf[:, k, :], hT[:, k, :])
                prev_hT_read[db] = reads
                for k in range(NK):
                    ps_h = ps_h0 if k < 4 else ps_h1
                    kk = k % 4
                    nc.tensor.transpose(ps_h[:, kk * 128:(kk + 1) * 128],
                                        hT[:, k, :], ident128[:])
                    nc.scalar.copy(h_aug[:, k * 128:(k + 1) * 128],
                                   ps_h[:, kk * 128:(kk + 1) * 128])

        nc.sync.dma_start(out[:], toks[:])

    nc.compile()
    return nc


class Runner:
    """Compile once; upload inputs and execute separately.

    Mirrors concourse.bass2jax.run_bass_via_pjrt's multi-core path, but
    keeps inputs device-resident so repeated execs measure device time
    rather than host->device transfer of ~250MB/core.
    """

    def __init__(self, nc, n_cores=NCORES):
        import jax
        from jax.experimental.shard_map import shard_map
        from jax.sharding import Mesh, PartitionSpec, NamedSharding
        from concourse import bass2jax as b2j
        from concourse import mybir

        b2j.install_neuronx_cc_hook()
        self.jax = jax
        self.nc = nc
        self.n_cores = n_cores
        partition_name = (
            nc.partition_id_tensor.name if nc.partition_id_tensor else None
        )
        in_names, out_names, out_avals, zero_outs = [], [], [], []
        for alloc in nc.m.functions[0].allocations:
            if not isinstance(alloc, mybir.MemoryLocationSet):
                continue
            name = alloc.memorylocations[0].name
            if alloc.kind == "ExternalInput":
                if name != partition_name:
                    in_names.append(name)
            elif alloc.kind == "ExternalOutput":
                shape = tuple(alloc.tensor_shape)
                dtype = mybir.dt.np(alloc.dtype)
                out_names.append(name)
                out_avals.append(jax.core.ShapedArray(shape, dtype))
                zero_outs.append(np.zeros(shape, dtype))
        n_params = len(in_names)
        n_outs = len(out_avals)
        all_in_names = list(in_names) + list(out_names)
        if partition_name is not None:
            all_in_names.append(partition_name)

        def _body(*args):
            operands = list(args)
            if partition_name is not None:
                operands.append(b2j.partition_id_tensor())
            outs = b2j._bass_exec_p.bind(
                *operands,
                out_avals=tuple(out_avals),
                in_names=tuple(all_in_names),
                out_names=tuple(out_names),
                lowering_input_output_aliases=(),
                sim_require_finite=True,
                sim_require_nnan=True,
                nc=nc,
            )
            return tuple(outs)

        devices = jax.devices()[:n_cores]
        assert len(devices) == n_cores, len(jax.devices())
        mesh = Mesh(np.asarray(devices), ("core",))
        in_specs = (PartitionSpec("core"),) * (n_params + n_outs)
        out_specs = (PartitionSpec("core"),) * n_outs
        self.sharded = jax.jit(
            shard_map(_body, mesh=mesh, in_specs=in_specs,
                      out_specs=out_specs, check_rep=False),
            donate_argnums=tuple(range(n_params, n_params + n_outs)),
            keep_unused=True,
        )
        self.sharding = NamedSharding(mesh, PartitionSpec("core"))
        self.in_names = in_names
        self.n_params = n_params
        self.out_names = out_names
        self.out_avals = out_avals
        self.zero_outs = zero_outs
        self.dev_in = None

    def upload(self, in_maps):
        concat = [
            np.concatenate(
                [np.asarray(m[name]) for m in in_maps], axis=0
            )
            for name in self.in_names
        ]
        self.dev_in = [self.jax.device_put(a, self.sharding) for a in concat]
        self.jax.block_until_ready(self.dev_in)

    def _zeros(self):
        return [
            self.jax.device_put(
                np.zeros((self.n_cores * z.shape[0], *z.shape[1:]), z.dtype),
                self.sharding,
            )
            for z in self.zero_outs
        ]

    def exec_async(self):
        return self.sharded(*self.dev_in, *self._zeros())

    def run(self):
        outs = self.exec_async()
        self.jax.block_until_ready(outs)
        return {
            name: np.asarray(outs[i]).reshape(
                self.n_cores, *self.out_avals[i].shape
            )
            for i, name in enumerate(self.out_names)
        }


_CACHE = {}


# build_v3 (k-outer screen + transposed ag1 transport) passes MultiCoreSim
# exactly but hits NRT_EXEC_UNIT_UNRECOVERABLE on real HW — keep the
# HW-verified v2 as the production build.
BUILD = build_v2


def get_runner():
    if "r" not in _CACHE:
        _CACHE["r"] = Runner(BUILD(T))
    return _CACHE["r"]


def kernel(**inputs):
    r = get_runner()
    r.upload(prep_inputs(**inputs))
    out = r.run()["out"]
    return np.asarray(out[0], dtype=np.int32)

